# revision 1
# baseline (speedup 1.0000x reference)
"""Cross-level attention (3 KV levels: causal T=2048, full T1=512, full T2=128)
for B=2, H=16, T=2048, DH=64 on 8 Trainium2 NeuronCores.

Sharding: the 32 (b, h) pairs are split 4-per-core (batch + head parallel);
each core computes full attention for its 4 heads, level_w replicated.

Per-core dataflow (all operands resident in SBUF after one initial load):
  - Heads are processed as 2 "duos" (2 pairs packed on partitions 0-63 /
    64-127) so the two K=64-contraction QK^T matmuls occupy disjoint PE row
    groups and run concurrently.
  - S^T tiles [s=128, t=512] are computed per 128-wide K-block j via
    matmul(lhsT=K^T, rhs=Q^T); exp (with the 1/sqrt(dh) scale folded in) runs
    on the Scalar engine straight out of PSUM into bf16 SBUF tiles P^T.
  - Causal masking for level 0 is block-level: fully-masked blocks are
    skipped (in both compute and exp), diagonal blocks get a 0/1
    upper-triangular multiply post-exp.
  - PV: matmul(lhsT=P^T[:, c-slice], rhs=V'[s-block]) accumulating in PSUM
    over s-blocks, where V' carries a ones-column so the softmax denominator
    accumulates in output column 64 for free. Level weights w_l are folded
    into V on the host, so combine is out = sum_l PV_l / rowsum_l
    (reciprocal + per-column broadcast multiply + adds on DVE).
  - The per-(duo, query-block) units are software-pipelined one deep
    (QK+exp of unit u before PV of unit u-1) so the Scalar engine — the
    bottleneck at ~1 exp/lane/cycle — never starves while the PE drains the
    PV backlog. Unit order and 512-column-chunked input DMAs minimize the
    startup and drain tails.

TimelineSim cost model: ~130 us/core (Scalar/exp-bound at ~116 us busy;
PE ~72 us modeled, DVE ~40 us, Pool ~13 us). HW-validated: rel-l2 2.8e-3.

Bottleneck notes (for future iteration): the hard floor is ~113M softmax
exponentials through the single exp-capable Scalar engine at 1 elem/lane/
cycle (~92 us elements + ~23 us per-call overhead). The per-call overhead
cannot be amortized further because exp reads PSUM and the 8-bank budget is
exactly consumed: 2x double-buffered [128,2,512] score tiles (4 banks) +
three per-level PV accumulators with handoff stagger (4 banks). Measured
dead ends: exp-call batching to 3 banks (needs 9-10 banks total), PV rowsum
via ones-rhs matmuls to free a bank (+13-16 us PE for -4 us ACT), masks or
combine-adds on Pool (regressed: they sit on the exp->PV critical path),
L2-first block order (delays the L0 PV stream feeding the pipeline, +3.8 us),
pool buffer increases (neutral). Remaining theoretical: ~5 us startup
(DMA-gated first exp) + ~7 us drain tail (last units' PV+combine), partially
irreducible without 2-3 extra PSUM banks for a fused final unit.
"""
import numpy as np
import ml_dtypes

B, H, T, DH = 2, 16, 2048, 64
C = H * DH
T1, T2 = 512, 128
NCORES = 8
PAIRS = 4          # (b, h) pairs per core
DUOS = 2           # pairs are packed two-per-SBUF-tile
QB = T // 512      # 512-wide query blocks
NJ = (T // 128, T1 // 128, T2 // 128)

TRACE = False          # set by test.py for profiling runs
LAST_RESULT = None     # BassKernelResults from the most recent run

_NC_CACHE = {}


def _build_nc(w):
    import concourse.bass as bass
    from concourse import bacc
    import concourse.tile as tile
    import concourse.mybir as mybir
    from contextlib import ExitStack

    BF16 = mybir.dt.bfloat16
    F32 = mybir.dt.float32
    Exp = mybir.ActivationFunctionType.Exp

    nc = bacc.Bacc("TRN2", target_bir_lowering=False)
    qt = nc.dram_tensor("qt", [DUOS, 128, T], BF16, kind="ExternalInput")
    k0t = nc.dram_tensor("k0t", [DUOS, 128, T], BF16, kind="ExternalInput")
    k1t = nc.dram_tensor("k1t", [DUOS, 128, T1], BF16, kind="ExternalInput")
    k2t = nc.dram_tensor("k2t", [DUOS, 128, T2], BF16, kind="ExternalInput")
    v0 = nc.dram_tensor("v0", [PAIRS, 128, NJ[0], 65], BF16, kind="ExternalInput")
    v1 = nc.dram_tensor("v1", [PAIRS, 128, NJ[1], 65], BF16, kind="ExternalInput")
    v2 = nc.dram_tensor("v2", [PAIRS, 128, NJ[2], 65], BF16, kind="ExternalInput")
    tri = nc.dram_tensor("tri", [128, 128], BF16, kind="ExternalInput")
    out = nc.dram_tensor("out", [PAIRS, 128, T // 128, DH], F32, kind="ExternalOutput")

    with tile.TileContext(nc) as tc, ExitStack() as ctx:
        const = ctx.enter_context(tc.tile_pool(name="const", bufs=1))
        qkp = ctx.enter_context(tc.tile_pool(name="qkp", bufs=2, space="PSUM"))
        pvp = ctx.enter_context(tc.tile_pool(name="pvp", bufs=4, space="PSUM"))
        pts = ctx.enter_context(tc.tile_pool(name="pts", bufs=44))
        outp = ctx.enter_context(tc.tile_pool(name="outp", bufs=4))
        small = ctx.enter_context(tc.tile_pool(name="small", bufs=8))

        def load(dram_ap, shape, tag):
            t = const.tile(shape, BF16, tag=tag, name=tag)
            nc.sync.dma_start(out=t, in_=dram_ap)
            return t

        # Big Q/K tensors are DMA'd in 512-column chunks so the first
        # query-block's matmuls can start before the full tensors land.
        def load_chunked(dram_ap, shape, tag, chunk=512):
            t = const.tile(shape, BF16, tag=tag, name=tag)
            for c0 in range(0, shape[1], chunk):
                nc.sync.dma_start(out=t[:, c0:c0 + chunk],
                                  in_=dram_ap[:, c0:c0 + chunk])
            return t

        # Emission order = need order: the first unit is (d=0, qb=0), which
        # needs only qt0/k0t0 column chunk 0 plus k1t/k2t of duo 0. V tiles go
        # on the gpsimd SWDGE queue so they stream in parallel with the
        # HWDGE-queued Q/K chunks.
        sb_tri = load(tri[:], [128, 128], "tri")
        def alloc(shape, tag):
            return const.tile(shape, BF16, tag=tag, name=tag)
        sb_qt = [alloc([128, T], f"qt{d}") for d in range(DUOS)]
        sb_kt = [[alloc([128, T], f"k0t{d}") for d in range(DUOS)],
                 [alloc([128, T1], f"k1t{d}") for d in range(DUOS)],
                 [alloc([128, T2], f"k2t{d}") for d in range(DUOS)]]
        sb_v = [[alloc([128, NJ[0], 65], f"v0{p}") for p in range(PAIRS)],
                [alloc([128, NJ[1], 65], f"v1{p}") for p in range(PAIRS)],
                [alloc([128, NJ[2], 65], f"v2{p}") for p in range(PAIRS)]]

        def dma_cols(t, dram_ap, c0, c1, engine=None):
            (engine or nc.sync).dma_start(out=t[:, c0:c1], in_=dram_ap[:, c0:c1])

        for d in range(DUOS):
            # unit (d, 0): first 512 cols of qt/k0t + all of k1t/k2t.
            # k0t's first chunk is split so j=0's 128 columns land first and
            # the very first QK matmul can issue as early as possible.
            dma_cols(sb_kt[0][d], k0t[d], 0, 128)
            dma_cols(sb_qt[d], qt[d], 0, 512)
            dma_cols(sb_kt[0][d], k0t[d], 128, 512)
            nc.sync.dma_start(out=sb_kt[1][d], in_=k1t[d])
            nc.sync.dma_start(out=sb_kt[2][d], in_=k2t[d])
            for p in (2 * d, 2 * d + 1):
                nc.gpsimd.dma_start(out=sb_v[0][p], in_=v0[p])
                nc.gpsimd.dma_start(out=sb_v[1][p], in_=v1[p])
                nc.gpsimd.dma_start(out=sb_v[2][p], in_=v2[p])
            # remaining qt/k0t chunks (needed from unit (d, 3) onwards)
            for c0 in range(512, T, 512):
                dma_cols(sb_kt[0][d], k0t[d], c0, c0 + 512)
            for c0 in range(512, T, 512):
                dma_cols(sb_qt[d], qt[d], c0, c0 + 512)

        def unit_jblocks(qb):
            return ([(0, j) for j in range(4 * qb + 4)]
                    + [(1, j) for j in range(NJ[1])]
                    + [(2, j) for j in range(NJ[2])])

        def phase1(d, qb):
                jblocks = unit_jblocks(qb)
                # ---- Phase 1: S^T = K^T.T @ Q^T per j-block, exp -> P^T
                pt_tiles = {}
                for (l, j) in jblocks:
                    # Diagonal L0 blocks: columns left of the diagonal 128-col
                    # sub-block are fully causal-masked and never read by PV,
                    # so compute/exp only the [f0, 512) column range.
                    f0 = 128 * (j - 4 * qb) if (l == 0 and j >= 4 * qb) else 0
                    sp = qkp.tile([128, 2, 512], F32, tag="qk", name="qk")
                    for half in range(2):
                        nc.tensor.matmul(
                            out=sp[:, half, f0:],
                            lhsT=sb_kt[l][d][64 * half:64 * half + 64,
                                             128 * j:128 * j + 128],
                            rhs=sb_qt[d][64 * half:64 * half + 64,
                                         512 * qb + f0:512 * qb + 512],
                            start=True, stop=True,
                        )
                    pt = pts.tile([128, 2, 512], BF16, tag="pt", name="pt")
                    nc.scalar.activation(out=pt[:, :, f0:], in_=sp[:, :, f0:],
                                         func=Exp, scale=DH ** -0.5)
                    if l == 0 and j >= 4 * qb:
                        for half in range(2):
                            s = pt[:, half, f0:f0 + 128]
                            nc.vector.tensor_mul(out=s, in0=s, in1=sb_tri)
                    pt_tiles[(l, j)] = pt
                return pt_tiles

        def phase2(d, qb, pt_tiles):
                jblocks = unit_jblocks(qb)
                # ---- Phase 2: PV accumulation + combine, per pair half
                for half in range(2):
                    p = 2 * d + half
                    pvt = [pvp.tile([128, 4, 65], F32, tag="pv", name=f"pv{_l}") for _l in range(3)]
                    for (l, j) in jblocks:
                        pt = pt_tiles[(l, j)]
                        # One accumulation group per PSUM bank (= per level):
                        # start zeroes the whole 2KB zero region, so only the
                        # very first matmul into the bank may set start=True.
                        lvl_last_j = (4 * qb + 3) if l == 0 else NJ[l] - 1
                        for c in range(4):
                            if l == 0 and j > 4 * qb + c:
                                continue
                            nc.tensor.matmul(
                                out=pvt[l][:, c, :],
                                lhsT=pt[:, half, 128 * c:128 * c + 128],
                                rhs=sb_v[l][p][:, j, :],
                                start=(j == 0 and c == 0),
                                stop=(j == lvl_last_j and c == 3),
                            )
                    osb = outp.tile([128, 4, DH], F32, tag="osb", name="osb")
                    for l in range(3):
                        rc = small.tile([128, 4, 1], F32, tag="rc", name="rc")
                        nc.vector.reciprocal(out=rc[:, :, 0], in_=pvt[l][:, :, 64])
                        dst = osb if l == 0 else outp.tile([128, 4, DH], F32, tag="tmp", name="tmp")
                        nc.vector.tensor_mul(
                            out=dst, in0=pvt[l][:, :, 0:64],
                            in1=rc.broadcast_to([128, 4, DH]))
                        if l > 0:
                            nc.vector.tensor_add(out=osb, in0=osb, in1=dst)
                    nc.sync.dma_start(out=out[p][:, 4 * qb:4 * qb + 4, :],
                                      in_=osb)

        # Software pipeline: run unit u's QK+exp before unit u-1's PV, so the
        # Scalar engine always has a full unit of exp work queued while the
        # PE drains the previous unit's PV backlog. Unit order puts a small
        # unit first (fast start after partial DMA) and a small-ish one last
        # (short PV tail after the final exp).
        units = [(0, 0), (0, 3), (0, 2), (0, 1), (1, 3), (1, 2), (1, 1), (1, 0)]
        tiles = {}
        pending = []
        for i, u in enumerate(units):
            tiles[u] = phase1(*u)
            if i < len(units) - 2:
                if pending:
                    v = pending.pop(0)
                    phase2(v[0], v[1], tiles.pop(v))
                pending.append(u)
            else:
                # run the last two units' QK+exp back-to-back so the Scalar
                # engine has a full runway while the PE drains the PV backlog
                pending.append(u)
        for v in pending:
            phase2(v[0], v[1], tiles.pop(v))
    nc.compile()
    return nc


def _prepare(inputs):
    bf = ml_dtypes.bfloat16
    Q = np.asarray(inputs["Q"], np.float32)
    Ks = [np.asarray(inputs[k], np.float32) for k in ("K0", "K1", "K2")]
    Vs = [np.asarray(inputs[k], np.float32) for k in ("V0", "V1", "V2")]
    level_w = np.asarray(inputs["level_w"], np.float64)
    e = np.exp(level_w - level_w.max())
    w = (e / e.sum()).astype(np.float64)

    # Host-side layout for sharding: per-head transposed Q/K ([64, Tm]) and
    # s-tiled V with a ones column ([128, nj, 65]).
    QT = np.ascontiguousarray(Q.transpose(0, 1, 3, 2)).astype(bf)  # [B,H,64,T]
    KTs = []
    for Kl in Ks:
        Tm = Kl.shape[1]
        Kh = Kl.reshape(B, Tm, H, DH).transpose(0, 2, 3, 1)  # [B,H,64,Tm]
        KTs.append(np.ascontiguousarray(Kh).astype(bf))
    Vps = []
    for lvl, Vl in enumerate(Vs):
        Tm = Vl.shape[1]
        Vl = Vl * np.float32(w[lvl])   # fold level weight into V (exact in fp32)
        Vh = Vl.reshape(B, Tm, H, DH).transpose(0, 2, 1, 3)  # [B,H,Tm,64]
        vp = np.ones((B, H, Tm // 128, 128, 65), np.float32)
        vp[..., :64] = Vh.reshape(B, H, Tm // 128, 128, DH)
        # -> [B, H, 128(p), nj, 65]
        Vps.append(np.ascontiguousarray(vp.transpose(0, 1, 3, 2, 4)).astype(bf))
    tri = (np.arange(128)[:, None] <= np.arange(128)[None, :]).astype(bf)

    in_maps = []
    for core in range(NCORES):
        m = {
            "qt": np.empty((DUOS, 128, T), bf),
            "k0t": np.empty((DUOS, 128, T), bf),
            "k1t": np.empty((DUOS, 128, T1), bf),
            "k2t": np.empty((DUOS, 128, T2), bf),
            "v0": np.empty((PAIRS, 128, NJ[0], 65), bf),
            "v1": np.empty((PAIRS, 128, NJ[1], 65), bf),
            "v2": np.empty((PAIRS, 128, NJ[2], 65), bf),
            "tri": tri,
        }
        for p in range(PAIRS):
            g = PAIRS * core + p
            b, h = divmod(g, H)
            d, half = divmod(p, 2)
            sl = slice(64 * half, 64 * half + 64)
            m["qt"][d, sl] = QT[b, h]
            m["k0t"][d, sl] = KTs[0][b, h]
            m["k1t"][d, sl] = KTs[1][b, h]
            m["k2t"][d, sl] = KTs[2][b, h]
            m["v0"][p] = Vps[0][b, h]
            m["v1"][p] = Vps[1][b, h]
            m["v2"][p] = Vps[2][b, h]
        in_maps.append(m)

    return in_maps, w


def kernel(**inputs):
    global LAST_RESULT
    from concourse.bass_utils import run_bass_kernel_spmd

    in_maps, w = _prepare(inputs)
    key = tuple(np.asarray(w, np.float64).tolist())
    if key not in _NC_CACHE:
        _NC_CACHE[key] = _build_nc(w)
    nc = _NC_CACHE[key]

    try:
        res = run_bass_kernel_spmd(nc, in_maps, core_ids=list(range(NCORES)),
                                   trace=TRACE)
    except (ImportError, ModuleNotFoundError):
        # axon build without the NTFF profiling hook — run without trace
        res = run_bass_kernel_spmd(nc, in_maps, core_ids=list(range(NCORES)),
                                   trace=False)
    LAST_RESULT = res

    outs = np.stack([np.asarray(r["out"]) for r in res.results])  # [8,4,128,16,64]
    O = outs.transpose(0, 1, 3, 2, 4).reshape(B, H, T, DH)  # t = 128*n + pp
    return np.ascontiguousarray(O.transpose(0, 2, 1, 3)).reshape(B, T, C).astype(np.float32)



# revision 27
# speedup vs baseline: 1.2328x; 1.2328x over previous
"""Cross-level attention (3 KV levels: causal T=2048, full T1=512, full T2=128)
for B=2, H=16, T=2048, DH=64 on 8 Trainium2 NeuronCores.

Sharding: the 32 (b, h) pairs are split 4-per-core (batch + head parallel);
each core computes full attention for its 4 heads, level_w replicated.

Per-core dataflow (all operands resident in SBUF after one initial load):
  - Heads are processed as 2 "duos" (2 pairs packed on partitions 0-63 /
    64-127) so the two K=64-contraction QK^T matmuls occupy disjoint PE row
    groups and run concurrently.
  - S^T tiles [s=128, t=512] are computed per 128-wide K-block j via
    matmul(lhsT=K^T, rhs=Q^T); exp (with the 1/sqrt(dh) scale folded in) runs
    on the Scalar engine straight out of PSUM into bf16 SBUF tiles P^T.
  - Causal masking for level 0 is block-level: fully-masked blocks are
    skipped (in both compute and exp), diagonal blocks get a 0/1
    upper-triangular multiply post-exp.
  - PV: matmul(lhsT=P^T[:, c-slice], rhs=V'[s-block]) accumulating in PSUM
    over s-blocks, where V' carries a ones-column so the softmax denominator
    accumulates in output column 64 for free. Level weights w_l are folded
    into V on the host, so combine is out = sum_l PV_l / rowsum_l
    (reciprocal + per-column broadcast multiply + adds on DVE).
  - The per-(duo, query-block) units are software-pipelined one deep
    (QK+exp of unit u before PV of unit u-1) so the Scalar engine — the
    bottleneck at ~1 exp/lane/cycle — never starves while the PE drains the
    PV backlog. Unit order and 512-column-chunked input DMAs minimize the
    startup and drain tails.

TimelineSim cost model: ~130 us/core (Scalar/exp-bound at ~116 us busy;
PE ~72 us modeled, DVE ~40 us, Pool ~13 us). HW-validated: rel-l2 2.8e-3.

Bottleneck notes (for future iteration): the hard floor is ~113M softmax
exponentials through the single exp-capable Scalar engine at 1 elem/lane/
cycle (~92 us elements + ~23 us per-call overhead). The per-call overhead
cannot be amortized further because exp reads PSUM and the 8-bank budget is
exactly consumed: 2x double-buffered [128,2,512] score tiles (4 banks) +
three per-level PV accumulators with handoff stagger (4 banks). Measured
dead ends: exp-call batching to 3 banks (needs 9-10 banks total), PV rowsum
via ones-rhs matmuls to free a bank (+13-16 us PE for -4 us ACT), masks or
combine-adds on Pool (regressed: they sit on the exp->PV critical path),
L2-first block order (delays the L0 PV stream feeding the pipeline, +3.8 us),
pool buffer increases (neutral). Remaining theoretical: ~5 us startup
(DMA-gated first exp) + ~7 us drain tail (last units' PV+combine), partially
irreducible without 2-3 extra PSUM banks for a fused final unit.
"""
import numpy as np
import ml_dtypes

B, H, T, DH = 2, 16, 2048, 64
C = H * DH
T1, T2 = 512, 128
NCORES = 8
PAIRS = 4          # (b, h) pairs per core
DUOS = 2           # pairs are packed two-per-SBUF-tile
QB = T // 512      # 512-wide query blocks
NJ = (T // 128, T1 // 128, T2 // 128)

TRACE = False          # set by test.py for profiling runs
LAST_RESULT = None     # BassKernelResults from the most recent run

# Schraudolph approximate-exp constants (DVE offload): bf16 bit pattern of
# exp(s/8) ~= int16(round(s * SCH_A + SCH_B)); bias picked for zero mean
# multiplicative error (sigma ~1.7%), applied only to low-error L0 blocks.
SCH_A = 128 * np.log2(np.e) * (DH ** -0.5)
SCH_B = 128 * (127.0 - 0.0397 / np.log(2))
# Of the half-blocks eligible for DVE offload, keep every DVE_SKIP-th on ACT
DVE_SKIP = 5
# phase2 generator yields to the interleaver every PV_CHUNK matmuls
PV_CHUNK = 8

_NC_CACHE = {}


def _build_nc(w):
    import concourse.bass as bass
    from concourse import bacc
    import concourse.tile as tile
    import concourse.mybir as mybir
    from contextlib import ExitStack

    BF16 = mybir.dt.bfloat16
    F32 = mybir.dt.float32
    I16 = mybir.dt.int16
    Exp = mybir.ActivationFunctionType.Exp
    Mult = mybir.AluOpType.mult
    Add = mybir.AluOpType.add

    nc = bacc.Bacc("TRN2", target_bir_lowering=False)
    qt = nc.dram_tensor("qt", [DUOS, 128, T], BF16, kind="ExternalInput")
    k0t = nc.dram_tensor("k0t", [DUOS, 128, T], BF16, kind="ExternalInput")
    k1t = nc.dram_tensor("k1t", [DUOS, 128, T1], BF16, kind="ExternalInput")
    k2t = nc.dram_tensor("k2t", [DUOS, 128, T2], BF16, kind="ExternalInput")
    v0 = nc.dram_tensor("v0", [128, PAIRS, NJ[0], 65], BF16, kind="ExternalInput")
    v1 = nc.dram_tensor("v1", [128, PAIRS, NJ[1], 65], BF16, kind="ExternalInput")
    v2 = nc.dram_tensor("v2", [128, PAIRS, NJ[2], 65], BF16, kind="ExternalInput")
    tri = nc.dram_tensor("tri", [128, 128], BF16, kind="ExternalInput")
    out = nc.dram_tensor("out", [PAIRS, 128, T // 128, DH], F32, kind="ExternalOutput")

    with tile.TileContext(nc) as tc, ExitStack() as ctx:
        const = ctx.enter_context(tc.tile_pool(name="const", bufs=1))
        qkp = ctx.enter_context(tc.tile_pool(name="qkp", bufs=5, space="PSUM"))
        pvp = ctx.enter_context(tc.tile_pool(name="pvp", bufs=3, space="PSUM"))
        pts = ctx.enter_context(tc.tile_pool(name="pts", bufs=88))
        outp = ctx.enter_context(tc.tile_pool(name="outp", bufs=4))
        small = ctx.enter_context(tc.tile_pool(name="small", bufs=8))

        def load(dram_ap, shape, tag):
            t = const.tile(shape, BF16, tag=tag, name=tag)
            nc.sync.dma_start(out=t, in_=dram_ap)
            return t

        # Big Q/K tensors are DMA'd in 512-column chunks so the first
        # query-block's matmuls can start before the full tensors land.
        def load_chunked(dram_ap, shape, tag, chunk=512):
            t = const.tile(shape, BF16, tag=tag, name=tag)
            for c0 in range(0, shape[1], chunk):
                nc.sync.dma_start(out=t[:, c0:c0 + chunk],
                                  in_=dram_ap[:, c0:c0 + chunk])
            return t

        # Emission order = need order: the first unit is (d=0, qb=0), which
        # needs only qt0/k0t0 column chunk 0 plus k1t/k2t of duo 0. V tiles go
        # on the gpsimd SWDGE queue so they stream in parallel with the
        # HWDGE-queued Q/K chunks.
        sb_tri = load(tri[:], [128, 128], "tri")
        def alloc(shape, tag):
            return const.tile(shape, BF16, tag=tag, name=tag)
        sb_qt = [alloc([128, T], f"qt{d}") for d in range(DUOS)]
        sb_kt = [[alloc([128, T], f"k0t{d}") for d in range(DUOS)],
                 [alloc([128, T1], f"k1t{d}") for d in range(DUOS)],
                 [alloc([128, T2], f"k2t{d}") for d in range(DUOS)]]
        # All pairs' V tiles for one level share a single SBUF tile; loaded
        # in per-duo SWDGE DMAs so no single transfer blocks the serial DMA
        # stream for long, ordered by first use.
        sb_v = [alloc([128, PAIRS, NJ[l], 65], f"v{l}") for l in range(3)]

        def dma_cols(t, dram_ap, c0, c1, engine=None):
            (engine or nc.sync).dma_start(out=t[:, c0:c1], in_=dram_ap[:, c0:c1])

        def v_load(l, d):
            dram = (v0, v1, v2)[l]
            nc.gpsimd.dma_start(out=sb_v[l][:, 2 * d:2 * d + 2],
                                in_=dram[:, 2 * d:2 * d + 2])

        # DMAs execute in issue order, so emit strictly by need time:
        # unit (0,0) slices, duo-0 V tiles, rest of duo 0 (qt high chunks
        # first: unit order is qb=3,2,1), then duo 1.
        dma_cols(sb_kt[0][0], k0t[0], 0, 128)
        dma_cols(sb_qt[0], qt[0], 0, 512)
        nc.sync.dma_start(out=sb_kt[1][0], in_=k1t[0])
        dma_cols(sb_kt[0][0], k0t[0], 128, 512)
        nc.sync.dma_start(out=sb_kt[2][0], in_=k2t[0])
        for l in range(3):
            v_load(l, 0)
        for c0 in range(512, T, 512):
            dma_cols(sb_kt[0][0], k0t[0], c0, c0 + 512)
        for c0 in (1536, 1024, 512):
            dma_cols(sb_qt[0], qt[0], c0, c0 + 512)
        # duo 1 (first needed at unit (1,3), mid-kernel)
        for c0 in range(0, T, 512):
            dma_cols(sb_kt[0][1], k0t[1], c0, c0 + 512)
        dma_cols(sb_qt[1], qt[1], 1536, 2048)
        for l in range(3):
            v_load(l, 1)
        for c0 in (1024, 512, 0):
            dma_cols(sb_qt[1], qt[1], c0, c0 + 512)
        nc.sync.dma_start(out=sb_kt[1][1], in_=k1t[1])
        nc.sync.dma_start(out=sb_kt[2][1], in_=k2t[1])

        def unit_jblocks(qb):
            return ([(0, j) for j in range(4 * qb + 4)]
                    + [(1, j) for j in range(NJ[1])]
                    + [(2, j) for j in range(NJ[2])])

        def unit_exp_order(qb):
            """Emission order + exp-engine assignment for one unit.

            L0 full blocks of qb>=1 go to DVE via Schraudolph approx exp
            (softmax weight noise ~1.7% there costs ~0.4% output rel-l2
            because those queries average over >=512 keys); qb=0 units
            offload their L1 blocks instead (keeps DVE busy at startup,
            ~0.3% rel-l2). Everything else stays exact on ACT. The two
            streams are interleaved so both engines ping-pong the 2 PSUM
            score buffers concurrently.
            """
            if qb >= 1:
                elig = [(0, j, h) for j in range(4 * qb) for h in range(2)]
            else:
                elig = [(1, j, h) for j in range(NJ[1]) for h in range(2)]
            dve = [it for i, it in enumerate(elig) if i % DVE_SKIP != DVE_SKIP - 1]
            dset = set(dve)
            act = [(l, j, h) for (l, j) in unit_jblocks(qb) for h in range(2)
                   if (l, j, h) not in dset]
            order = []
            na, nd = len(act), len(dve)
            ia = id_ = 0
            for _ in range(na + nd):
                # Bresenham-style proportional interleave of the two streams
                if id_ < nd and (ia >= na or id_ * na < ia * nd):
                    order.append((dve[id_], "V")); id_ += 1
                else:
                    order.append((act[ia], "A")); ia += 1
            return order

        def emit_block(d, qb, l, j, half, eng, pt_tiles):
                # ---- Phase 1 step: S^T = K^T.T @ Q^T for one (j, half),
                # then exp (ACT exact / DVE Schraudolph) -> P^T. One PSUM
                # bank per tile so 4 score buffers circulate concurrently.
                # Diagonal L0 blocks: columns left of the diagonal 128-col
                # sub-block are fully causal-masked and never read by PV,
                # so compute/exp only the [f0, 512) column range.
                f0 = 128 * (j - 4 * qb) if (l == 0 and j >= 4 * qb) else 0
                sp = qkp.tile([128, 512], F32, tag="qk", name="qk")
                nc.tensor.matmul(
                    out=sp[:, f0:],
                    lhsT=sb_kt[l][d][64 * half:64 * half + 64,
                                     128 * j:128 * j + 128],
                    rhs=sb_qt[d][64 * half:64 * half + 64,
                                 512 * qb + f0:512 * qb + 512],
                    start=True, stop=True,
                )
                pt = pts.tile([128, 512], BF16, tag="pt", name="pt")
                if eng == "V":
                    nc.vector.tensor_scalar(
                        out=pt.bitcast(I16), in0=sp,
                        scalar1=float(SCH_A), scalar2=float(SCH_B),
                        op0=Mult, op1=Add)
                else:
                    nc.scalar.activation(out=pt[:, f0:], in_=sp[:, f0:],
                                         func=Exp, scale=DH ** -0.5)
                if l == 0 and j >= 4 * qb:
                    s = pt[:, f0:f0 + 128]
                    nc.gpsimd.tensor_mul(out=s, in0=s, in1=sb_tri)
                pt_tiles[(l, j, half)] = pt

        def phase2_gen(d, qb, pt_tiles):
                jblocks = unit_jblocks(qb)
                # ---- Phase 2: PV accumulation + combine, per pair half.
                # A generator yielding every few matmuls so the driver can
                # interleave this PE-ready work between the next unit's
                # exp-gated QK blocks (keeps the in-order PE stream busy
                # while the exp engines drain).
                for half in range(2):
                    p = 2 * d + half
                    pvt = [pvp.tile([128, 4, 65], F32, tag="pv", name=f"pv{_l}") for _l in range(3)]
                    cnt = 0
                    for (l, j) in jblocks:
                        pt = pt_tiles[(l, j, half)]
                        # One accumulation group per PSUM bank (= per level):
                        # start zeroes the whole 2KB zero region, so only the
                        # very first matmul into the bank may set start=True.
                        lvl_last_j = (4 * qb + 3) if l == 0 else NJ[l] - 1
                        for c in range(4):
                            if l == 0 and j > 4 * qb + c:
                                continue
                            nc.tensor.matmul(
                                out=pvt[l][:, c, :],
                                lhsT=pt[:, 128 * c:128 * c + 128],
                                rhs=sb_v[l][:, p, j, :],
                                start=(j == 0 and c == 0),
                                stop=(j == lvl_last_j and c == 3),
                            )
                            cnt += 1
                            if cnt >= PV_CHUNK:
                                cnt = 0
                                yield
                    osb = outp.tile([128, 4, DH], F32, tag="osb", name="osb")
                    for l in range(3):
                        rc = small.tile([128, 4, 1], F32, tag="rc", name="rc")
                        nc.vector.reciprocal(out=rc[:, :, 0], in_=pvt[l][:, :, 64])
                        dst = osb if l == 0 else outp.tile([128, 4, DH], F32, tag="tmp", name="tmp")
                        nc.vector.tensor_mul(
                            out=dst, in0=pvt[l][:, :, 0:64],
                            in1=rc.broadcast_to([128, 4, DH]))
                        if l > 0:
                            nc.gpsimd.tensor_add(out=osb, in0=osb, in1=dst)
                    nc.sync.dma_start(out=out[p][:, 4 * qb:4 * qb + 4, :],
                                      in_=osb)
                    yield

        # Software pipeline: unit u's PV/combine stream is emitted interleaved
        # between unit u+1's QK+exp blocks (one generator chunk per block), so
        # the in-order PE always has satisfiable PV work queued ahead of each
        # exp-gated QK matmul. Unit order puts a small unit first (fast start
        # after partial DMA) and a small one last (short PV tail).
        units = [(0, 0), (0, 3), (0, 2), (0, 1), (1, 3), (1, 2), (1, 1), (1, 0)]
        gen = None
        for u in units[:-1]:
            d, qb = u
            tiles = {}
            for ((l, j, half), eng) in unit_exp_order(qb):
                emit_block(d, qb, l, j, half, eng, tiles)
                if gen is not None:
                    next(gen, None)
            if gen is not None:
                for _ in gen:
                    pass
            gen = phase2_gen(d, qb, tiles)
        # Last unit: emit half 0's blocks first, then start its own PV
        # generator during half 1's blocks (safe pumps only reach half-0
        # PV work), trimming the drain tail to half 1's PV + combine.
        d, qb = units[-1]
        tiles = {}
        order = unit_exp_order(qb)
        h0 = [it for it in order if it[0][2] == 0]
        h1 = [it for it in order if it[0][2] == 1]
        for ((l, j, half), eng) in h0:
            emit_block(d, qb, l, j, half, eng, tiles)
            if gen is not None:
                next(gen, None)
        for _ in gen:
            pass
        gen = phase2_gen(d, qb, tiles)
        safe = (16 * qb + 30) // PV_CHUNK + 1
        for i, ((l, j, half), eng) in enumerate(h1):
            emit_block(d, qb, l, j, half, eng, tiles)
            if safe > 0:
                next(gen, None)
                safe -= 1
        for _ in gen:
            pass
    nc.compile()
    return nc


def _prepare(inputs):
    bf = ml_dtypes.bfloat16
    Q = np.asarray(inputs["Q"], np.float32)
    Ks = [np.asarray(inputs[k], np.float32) for k in ("K0", "K1", "K2")]
    Vs = [np.asarray(inputs[k], np.float32) for k in ("V0", "V1", "V2")]
    level_w = np.asarray(inputs["level_w"], np.float64)
    e = np.exp(level_w - level_w.max())
    w = (e / e.sum()).astype(np.float64)

    # Host-side layout for sharding: per-head transposed Q/K ([64, Tm]) and
    # s-tiled V with a ones column ([128, nj, 65]).
    QT = np.ascontiguousarray(Q.transpose(0, 1, 3, 2)).astype(bf)  # [B,H,64,T]
    KTs = []
    for Kl in Ks:
        Tm = Kl.shape[1]
        Kh = Kl.reshape(B, Tm, H, DH).transpose(0, 2, 3, 1)  # [B,H,64,Tm]
        KTs.append(np.ascontiguousarray(Kh).astype(bf))
    Vps = []
    for lvl, Vl in enumerate(Vs):
        Tm = Vl.shape[1]
        Vl = Vl * np.float32(w[lvl])   # fold level weight into V (exact in fp32)
        Vh = Vl.reshape(B, Tm, H, DH).transpose(0, 2, 1, 3)  # [B,H,Tm,64]
        vp = np.ones((B, H, Tm // 128, 128, 65), np.float32)
        vp[..., :64] = Vh.reshape(B, H, Tm // 128, 128, DH)
        # -> [B, H, 128(p), nj, 65]
        Vps.append(np.ascontiguousarray(vp.transpose(0, 1, 3, 2, 4)).astype(bf))
    tri = (np.arange(128)[:, None] <= np.arange(128)[None, :]).astype(bf)

    in_maps = []
    for core in range(NCORES):
        m = {
            "qt": np.empty((DUOS, 128, T), bf),
            "k0t": np.empty((DUOS, 128, T), bf),
            "k1t": np.empty((DUOS, 128, T1), bf),
            "k2t": np.empty((DUOS, 128, T2), bf),
            "v0": np.empty((128, PAIRS, NJ[0], 65), bf),
            "v1": np.empty((128, PAIRS, NJ[1], 65), bf),
            "v2": np.empty((128, PAIRS, NJ[2], 65), bf),
            "tri": tri,
        }
        for p in range(PAIRS):
            g = PAIRS * core + p
            b, h = divmod(g, H)
            d, half = divmod(p, 2)
            sl = slice(64 * half, 64 * half + 64)
            m["qt"][d, sl] = QT[b, h]
            m["k0t"][d, sl] = KTs[0][b, h]
            m["k1t"][d, sl] = KTs[1][b, h]
            m["k2t"][d, sl] = KTs[2][b, h]
            m["v0"][:, p] = Vps[0][b, h]
            m["v1"][:, p] = Vps[1][b, h]
            m["v2"][:, p] = Vps[2][b, h]
        in_maps.append(m)

    return in_maps, w


def kernel(**inputs):
    global LAST_RESULT
    from concourse.bass_utils import run_bass_kernel_spmd

    in_maps, w = _prepare(inputs)
    key = tuple(np.asarray(w, np.float64).tolist())
    if key not in _NC_CACHE:
        _NC_CACHE[key] = _build_nc(w)
    nc = _NC_CACHE[key]

    try:
        res = run_bass_kernel_spmd(nc, in_maps, core_ids=list(range(NCORES)),
                                   trace=TRACE)
    except (ImportError, ModuleNotFoundError):
        # axon build without the NTFF profiling hook — run without trace
        res = run_bass_kernel_spmd(nc, in_maps, core_ids=list(range(NCORES)),
                                   trace=False)
    LAST_RESULT = res

    outs = np.stack([np.asarray(r["out"]) for r in res.results])  # [8,4,128,16,64]
    O = outs.transpose(0, 1, 3, 2, 4).reshape(B, H, T, DH)  # t = 128*n + pp
    return np.ascontiguousarray(O.transpose(0, 2, 1, 3)).reshape(B, T, C).astype(np.float32)



# revision 33
# speedup vs baseline: 1.2704x; 1.0305x over previous
"""Cross-level attention (3 KV levels: causal T=2048, full T1=512, full T2=128)
for B=2, H=16, T=2048, DH=64 on 8 Trainium2 NeuronCores.

Sharding: the 32 (b, h) pairs are split 4-per-core (batch + head parallel);
each core computes full attention for its 4 heads, level_w replicated.

Per-core dataflow (all operands resident in SBUF after one initial load):
  - Heads are processed as 2 "duos" (2 pairs packed on partitions 0-63 /
    64-127); per (duo, 512-query block, 128-key block j, half) one QK^T
    matmul produces an S^T tile [s=128, t=512] in a single PSUM bank.
  - exp runs SPLIT ACROSS TWO ENGINES: ACT computes exact exp (scale folded
    in) for the error-sensitive blocks (qb=0, L2, L0 diagonals, and a 1/5
    keep-share); DVE computes a Schraudolph approximate exp for the rest --
    one tensor_scalar affine (s*SCH_A + SCH_B) written as int16 = the bf16
    BIT PATTERN of exp(s/8) (sigma ~1.7% multiplicative noise). Offloaded
    blocks are chosen where softmax averages over >=512 keys, so measured
    output rel-l2 only rises 2.8e-3 -> 4.8e-3 (gate 2e-2).
  - 5 one-bank score buffers circulate: the serial loop exp(k) -> free ->
    QK(k+2) -> exp(k+2) limits exp throughput to bufs/roundtrip; 5 bufs x
    ~1.3us roundtrip sustains the needed ~1 exp/350ns across both engines.
  - Causal masking for level 0 is block-level: fully-masked blocks skipped,
    diagonal blocks get a 0/1 upper-tri multiply post-exp (on Pool).
  - PV: matmul(lhsT=P^T[c-slice], rhs=V'[s-block]) accumulating in 3 PSUM
    banks (one per level), V' carries a ones-column so the denominator
    lands in column 64. Combine = sum_l PV_l * recip(rowsum_l): recip+mul
    on DVE (PSUM-capable), cross-level adds on Pool (SBUF-only engine).
  - The unit pipeline interleaves EMISSION: unit u's PV/combine generator is
    pumped one chunk (PV_CHUNK matmuls) per exp block of unit u+1, so the
    in-order PE always has satisfiable PV work queued ahead of exp-gated QK
    matmuls. The last unit splits halves so its own PV overlaps its phase 1.
  - DMAs execute in issue order, so they are emitted strictly by need time
    (first unit's Q/K slices, duo-0 V, duo-0 remainder, duo-1); V tiles ride
    the Pool SWDGE queue in per-duo transfers.

TimelineSim cost model: ~102.3 us/core (DVE 85.5 busy / ACT 81.6 / PE 71.6 /
Pool 48.3; ~6.5us DMA-gated startup + ~4us drain tail). CoreSim-validated:
rel-l2 4.8e-3. Down from 130 us for the single-exp-engine ancestor.

Bottleneck notes: exp element throughput is now jointly ACT+DVE-bound
(~167us of exp+combine engine work over two engines); PE sits at 71.6us.
Pool CANNOT read PSUM (BIR verifier) so it can't take exp or combine-mul
work, and dma_start cannot read PSUM either (no staging path) -- Pool is
capped at tri masks + combine adds + V DMAs. Measured dead ends: 2-bank
score tiles with 2 bufs (buffer roundtrip caps exp rate at ~1/950ns),
fp8/DoubleRow QK or PV (quantization alone costs ~2% rel-l2), consolidating
all V DMAs into one transfer (3us serial-DMA blob delays first QK by 2.5us).
"""
import numpy as np
import ml_dtypes

B, H, T, DH = 2, 16, 2048, 64
C = H * DH
T1, T2 = 512, 128
NCORES = 8
PAIRS = 4          # (b, h) pairs per core
DUOS = 2           # pairs are packed two-per-SBUF-tile
QB = T // 512      # 512-wide query blocks
NJ = (T // 128, T1 // 128, T2 // 128)

TRACE = False          # set by test.py for profiling runs
LAST_RESULT = None     # BassKernelResults from the most recent run

# Schraudolph approximate-exp constants (DVE offload): bf16 bit pattern of
# exp(s/8) ~= int16(round(s * SCH_A + SCH_B)); bias picked for zero mean
# multiplicative error (sigma ~1.7%), applied only to low-error L0 blocks.
SCH_A = 128 * np.log2(np.e) * (DH ** -0.5)
SCH_B = 128 * (127.0 - 0.0397 / np.log(2))
# Of the half-blocks eligible for DVE offload, keep every DVE_SKIP-th on ACT
DVE_SKIP = 5
# phase2 generator yields to the interleaver every PV_CHUNK matmuls
PV_CHUNK = 5

_NC_CACHE = {}


def _build_nc(w):
    import concourse.bass as bass
    from concourse import bacc
    import concourse.tile as tile
    import concourse.mybir as mybir
    from contextlib import ExitStack

    BF16 = mybir.dt.bfloat16
    F32 = mybir.dt.float32
    I16 = mybir.dt.int16
    Exp = mybir.ActivationFunctionType.Exp
    Mult = mybir.AluOpType.mult
    Add = mybir.AluOpType.add

    nc = bacc.Bacc("TRN2", target_bir_lowering=False)
    qt = nc.dram_tensor("qt", [DUOS, 128, T], BF16, kind="ExternalInput")
    k0t = nc.dram_tensor("k0t", [DUOS, 128, T], BF16, kind="ExternalInput")
    k1t = nc.dram_tensor("k1t", [DUOS, 128, T1], BF16, kind="ExternalInput")
    k2t = nc.dram_tensor("k2t", [DUOS, 128, T2], BF16, kind="ExternalInput")
    v0 = nc.dram_tensor("v0", [128, PAIRS, NJ[0], 65], BF16, kind="ExternalInput")
    v1 = nc.dram_tensor("v1", [128, PAIRS, NJ[1], 65], BF16, kind="ExternalInput")
    v2 = nc.dram_tensor("v2", [128, PAIRS, NJ[2], 65], BF16, kind="ExternalInput")
    tri = nc.dram_tensor("tri", [128, 128], BF16, kind="ExternalInput")
    out = nc.dram_tensor("out", [PAIRS, 128, T // 128, DH], F32, kind="ExternalOutput")

    with tile.TileContext(nc) as tc, ExitStack() as ctx:
        const = ctx.enter_context(tc.tile_pool(name="const", bufs=1))
        qkp = ctx.enter_context(tc.tile_pool(name="qkp", bufs=5, space="PSUM"))
        pvp = ctx.enter_context(tc.tile_pool(name="pvp", bufs=3, space="PSUM"))
        pts = ctx.enter_context(tc.tile_pool(name="pts", bufs=88))
        outp = ctx.enter_context(tc.tile_pool(name="outp", bufs=4))
        small = ctx.enter_context(tc.tile_pool(name="small", bufs=8))

        def load(dram_ap, shape, tag):
            t = const.tile(shape, BF16, tag=tag, name=tag)
            nc.sync.dma_start(out=t, in_=dram_ap)
            return t

        # Big Q/K tensors are DMA'd in 512-column chunks so the first
        # query-block's matmuls can start before the full tensors land.
        def load_chunked(dram_ap, shape, tag, chunk=512):
            t = const.tile(shape, BF16, tag=tag, name=tag)
            for c0 in range(0, shape[1], chunk):
                nc.sync.dma_start(out=t[:, c0:c0 + chunk],
                                  in_=dram_ap[:, c0:c0 + chunk])
            return t

        # Emission order = need order: the first unit is (d=0, qb=0), which
        # needs only qt0/k0t0 column chunk 0 plus k1t/k2t of duo 0. V tiles go
        # on the gpsimd SWDGE queue so they stream in parallel with the
        # HWDGE-queued Q/K chunks.
        sb_tri = load(tri[:], [128, 128], "tri")
        def alloc(shape, tag):
            return const.tile(shape, BF16, tag=tag, name=tag)
        sb_qt = [alloc([128, T], f"qt{d}") for d in range(DUOS)]
        sb_kt = [[alloc([128, T], f"k0t{d}") for d in range(DUOS)],
                 [alloc([128, T1], f"k1t{d}") for d in range(DUOS)],
                 [alloc([128, T2], f"k2t{d}") for d in range(DUOS)]]
        # All pairs' V tiles for one level share a single SBUF tile; loaded
        # in per-duo SWDGE DMAs so no single transfer blocks the serial DMA
        # stream for long, ordered by first use.
        sb_v = [alloc([128, PAIRS, NJ[l], 65], f"v{l}") for l in range(3)]

        def dma_cols(t, dram_ap, c0, c1, engine=None):
            (engine or nc.sync).dma_start(out=t[:, c0:c1], in_=dram_ap[:, c0:c1])

        def v_load(l, d):
            dram = (v0, v1, v2)[l]
            nc.gpsimd.dma_start(out=sb_v[l][:, 2 * d:2 * d + 2],
                                in_=dram[:, 2 * d:2 * d + 2])

        # DMAs execute in issue order, so emit strictly by need time:
        # unit (0,0) slices, duo-0 V tiles, rest of duo 0 (qt high chunks
        # first: unit order is qb=3,2,1), then duo 1.
        dma_cols(sb_kt[0][0], k0t[0], 0, 128)
        dma_cols(sb_qt[0], qt[0], 0, 512)
        nc.sync.dma_start(out=sb_kt[1][0], in_=k1t[0])
        dma_cols(sb_kt[0][0], k0t[0], 128, 512)
        nc.sync.dma_start(out=sb_kt[2][0], in_=k2t[0])
        for l in range(3):
            v_load(l, 0)
        for c0 in range(512, T, 512):
            dma_cols(sb_kt[0][0], k0t[0], c0, c0 + 512)
        for c0 in (1536, 1024, 512):
            dma_cols(sb_qt[0], qt[0], c0, c0 + 512)
        # duo 1 (first needed at unit (1,3), mid-kernel)
        for c0 in range(0, T, 512):
            dma_cols(sb_kt[0][1], k0t[1], c0, c0 + 512)
        dma_cols(sb_qt[1], qt[1], 1536, 2048)
        for l in range(3):
            v_load(l, 1)
        for c0 in (1024, 512, 0):
            dma_cols(sb_qt[1], qt[1], c0, c0 + 512)
        nc.sync.dma_start(out=sb_kt[1][1], in_=k1t[1])
        nc.sync.dma_start(out=sb_kt[2][1], in_=k2t[1])

        def unit_jblocks(qb):
            return ([(0, j) for j in range(4 * qb + 4)]
                    + [(1, j) for j in range(NJ[1])]
                    + [(2, j) for j in range(NJ[2])])

        def unit_exp_order(qb):
            """Emission order + exp-engine assignment for one unit.

            L0 full blocks of qb>=1 go to DVE via Schraudolph approx exp
            (softmax weight noise ~1.7% there costs ~0.4% output rel-l2
            because those queries average over >=512 keys); qb=0 units
            offload their L1 blocks instead (keeps DVE busy at startup,
            ~0.3% rel-l2). Everything else stays exact on ACT. The two
            streams are interleaved so both engines ping-pong the 2 PSUM
            score buffers concurrently.
            """
            if qb >= 1:
                elig = [(0, j, h) for j in range(4 * qb) for h in range(2)]
            else:
                elig = [(1, j, h) for j in range(NJ[1]) for h in range(2)]
            skip = DVE_SKIP if qb >= 1 else 3
            dve = [it for i, it in enumerate(elig) if i % skip != skip - 1]
            dset = set(dve)
            act = [(l, j, h) for (l, j) in unit_jblocks(qb) for h in range(2)
                   if (l, j, h) not in dset]
            order = []
            na, nd = len(act), len(dve)
            ia = id_ = 0
            for _ in range(na + nd):
                # Bresenham-style proportional interleave of the two streams
                if id_ < nd and (ia >= na or id_ * na < ia * nd):
                    order.append((dve[id_], "V")); id_ += 1
                else:
                    order.append((act[ia], "A")); ia += 1
            return order

        def emit_block(d, qb, l, j, half, eng, pt_tiles):
                # ---- Phase 1 step: S^T = K^T.T @ Q^T for one (j, half),
                # then exp (ACT exact / DVE Schraudolph) -> P^T. One PSUM
                # bank per tile so 4 score buffers circulate concurrently.
                # Diagonal L0 blocks: columns left of the diagonal 128-col
                # sub-block are fully causal-masked and never read by PV,
                # so compute/exp only the [f0, 512) column range.
                f0 = 128 * (j - 4 * qb) if (l == 0 and j >= 4 * qb) else 0
                sp = qkp.tile([128, 512], F32, tag="qk", name="qk")
                nc.tensor.matmul(
                    out=sp[:, f0:],
                    lhsT=sb_kt[l][d][64 * half:64 * half + 64,
                                     128 * j:128 * j + 128],
                    rhs=sb_qt[d][64 * half:64 * half + 64,
                                 512 * qb + f0:512 * qb + 512],
                    start=True, stop=True,
                )
                pt = pts.tile([128, 512], BF16, tag="pt", name="pt")
                if eng == "V":
                    nc.vector.tensor_scalar(
                        out=pt.bitcast(I16), in0=sp,
                        scalar1=float(SCH_A), scalar2=float(SCH_B),
                        op0=Mult, op1=Add)
                else:
                    nc.scalar.activation(out=pt[:, f0:], in_=sp[:, f0:],
                                         func=Exp, scale=DH ** -0.5)
                if l == 0 and j >= 4 * qb:
                    s = pt[:, f0:f0 + 128]
                    nc.gpsimd.tensor_mul(out=s, in0=s, in1=sb_tri)
                pt_tiles[(l, j, half)] = pt

        def phase2_gen(d, qb, pt_tiles):
                jblocks = unit_jblocks(qb)
                # ---- Phase 2: PV accumulation + combine, per pair half.
                # A generator yielding every few matmuls so the driver can
                # interleave this PE-ready work between the next unit's
                # exp-gated QK blocks (keeps the in-order PE stream busy
                # while the exp engines drain).
                for half in range(2):
                    p = 2 * d + half
                    pvt = [pvp.tile([128, 4, 65], F32, tag="pv", name=f"pv{_l}") for _l in range(3)]
                    cnt = 0
                    for (l, j) in jblocks:
                        pt = pt_tiles[(l, j, half)]
                        # One accumulation group per PSUM bank (= per level):
                        # start zeroes the whole 2KB zero region, so only the
                        # very first matmul into the bank may set start=True.
                        lvl_last_j = (4 * qb + 3) if l == 0 else NJ[l] - 1
                        for c in range(4):
                            if l == 0 and j > 4 * qb + c:
                                continue
                            nc.tensor.matmul(
                                out=pvt[l][:, c, :],
                                lhsT=pt[:, 128 * c:128 * c + 128],
                                rhs=sb_v[l][:, p, j, :],
                                start=(j == 0 and c == 0),
                                stop=(j == lvl_last_j and c == 3),
                            )
                            cnt += 1
                            if cnt >= PV_CHUNK:
                                cnt = 0
                                yield
                    osb = outp.tile([128, 4, DH], F32, tag="osb", name="osb")
                    for l in range(3):
                        rc = small.tile([128, 4, 1], F32, tag="rc", name="rc")
                        nc.vector.reciprocal(out=rc[:, :, 0], in_=pvt[l][:, :, 64])
                        dst = osb if l == 0 else outp.tile([128, 4, DH], F32, tag="tmp", name="tmp")
                        nc.vector.tensor_mul(
                            out=dst, in0=pvt[l][:, :, 0:64],
                            in1=rc.broadcast_to([128, 4, DH]))
                        if l > 0:
                            nc.gpsimd.tensor_add(out=osb, in0=osb, in1=dst)
                    nc.sync.dma_start(out=out[p][:, 4 * qb:4 * qb + 4, :],
                                      in_=osb)
                    yield

        # Software pipeline: unit u's PV/combine stream is emitted interleaved
        # between unit u+1's QK+exp blocks (one generator chunk per block), so
        # the in-order PE always has satisfiable PV work queued ahead of each
        # exp-gated QK matmul. Unit order puts a small unit first (fast start
        # after partial DMA) and a small one last (short PV tail).
        units = [(0, 0), (0, 3), (0, 2), (0, 1), (1, 3), (1, 2), (1, 1), (1, 0)]
        gen = None
        for u in units[:-1]:
            d, qb = u
            tiles = {}
            for ((l, j, half), eng) in unit_exp_order(qb):
                emit_block(d, qb, l, j, half, eng, tiles)
                if gen is not None:
                    next(gen, None)
            if gen is not None:
                for _ in gen:
                    pass
            gen = phase2_gen(d, qb, tiles)
        # Last unit: emit half 0's blocks first, then start its own PV
        # generator during half 1's blocks (safe pumps only reach half-0
        # PV work), trimming the drain tail to half 1's PV + combine.
        d, qb = units[-1]
        tiles = {}
        order = unit_exp_order(qb)
        h0 = [it for it in order if it[0][2] == 0]
        h1 = [it for it in order if it[0][2] == 1]
        for ((l, j, half), eng) in h0:
            emit_block(d, qb, l, j, half, eng, tiles)
            if gen is not None:
                next(gen, None)
        for _ in gen:
            pass
        gen = phase2_gen(d, qb, tiles)
        safe = (16 * qb + 30) // PV_CHUNK + 1
        for i, ((l, j, half), eng) in enumerate(h1):
            emit_block(d, qb, l, j, half, eng, tiles)
            if safe > 0:
                next(gen, None)
                safe -= 1
        for _ in gen:
            pass
    nc.compile()
    return nc


def _prepare(inputs):
    bf = ml_dtypes.bfloat16
    Q = np.asarray(inputs["Q"], np.float32)
    Ks = [np.asarray(inputs[k], np.float32) for k in ("K0", "K1", "K2")]
    Vs = [np.asarray(inputs[k], np.float32) for k in ("V0", "V1", "V2")]
    level_w = np.asarray(inputs["level_w"], np.float64)
    e = np.exp(level_w - level_w.max())
    w = (e / e.sum()).astype(np.float64)

    # Host-side layout for sharding: per-head transposed Q/K ([64, Tm]) and
    # s-tiled V with a ones column ([128, nj, 65]).
    QT = np.ascontiguousarray(Q.transpose(0, 1, 3, 2)).astype(bf)  # [B,H,64,T]
    KTs = []
    for Kl in Ks:
        Tm = Kl.shape[1]
        Kh = Kl.reshape(B, Tm, H, DH).transpose(0, 2, 3, 1)  # [B,H,64,Tm]
        KTs.append(np.ascontiguousarray(Kh).astype(bf))
    Vps = []
    for lvl, Vl in enumerate(Vs):
        Tm = Vl.shape[1]
        Vl = Vl * np.float32(w[lvl])   # fold level weight into V (exact in fp32)
        Vh = Vl.reshape(B, Tm, H, DH).transpose(0, 2, 1, 3)  # [B,H,Tm,64]
        vp = np.ones((B, H, Tm // 128, 128, 65), np.float32)
        vp[..., :64] = Vh.reshape(B, H, Tm // 128, 128, DH)
        # -> [B, H, 128(p), nj, 65]
        Vps.append(np.ascontiguousarray(vp.transpose(0, 1, 3, 2, 4)).astype(bf))
    tri = (np.arange(128)[:, None] <= np.arange(128)[None, :]).astype(bf)

    in_maps = []
    for core in range(NCORES):
        m = {
            "qt": np.empty((DUOS, 128, T), bf),
            "k0t": np.empty((DUOS, 128, T), bf),
            "k1t": np.empty((DUOS, 128, T1), bf),
            "k2t": np.empty((DUOS, 128, T2), bf),
            "v0": np.empty((128, PAIRS, NJ[0], 65), bf),
            "v1": np.empty((128, PAIRS, NJ[1], 65), bf),
            "v2": np.empty((128, PAIRS, NJ[2], 65), bf),
            "tri": tri,
        }
        for p in range(PAIRS):
            g = PAIRS * core + p
            b, h = divmod(g, H)
            d, half = divmod(p, 2)
            sl = slice(64 * half, 64 * half + 64)
            m["qt"][d, sl] = QT[b, h]
            m["k0t"][d, sl] = KTs[0][b, h]
            m["k1t"][d, sl] = KTs[1][b, h]
            m["k2t"][d, sl] = KTs[2][b, h]
            m["v0"][:, p] = Vps[0][b, h]
            m["v1"][:, p] = Vps[1][b, h]
            m["v2"][:, p] = Vps[2][b, h]
        in_maps.append(m)

    return in_maps, w


def kernel(**inputs):
    global LAST_RESULT
    from concourse.bass_utils import run_bass_kernel_spmd

    in_maps, w = _prepare(inputs)
    key = tuple(np.asarray(w, np.float64).tolist())
    if key not in _NC_CACHE:
        _NC_CACHE[key] = _build_nc(w)
    nc = _NC_CACHE[key]

    try:
        res = run_bass_kernel_spmd(nc, in_maps, core_ids=list(range(NCORES)),
                                   trace=TRACE)
    except (ImportError, ModuleNotFoundError):
        # axon build without the NTFF profiling hook — run without trace
        res = run_bass_kernel_spmd(nc, in_maps, core_ids=list(range(NCORES)),
                                   trace=False)
    LAST_RESULT = res

    outs = np.stack([np.asarray(r["out"]) for r in res.results])  # [8,4,128,16,64]
    O = outs.transpose(0, 1, 3, 2, 4).reshape(B, H, T, DH)  # t = 128*n + pp
    return np.ascontiguousarray(O.transpose(0, 2, 1, 3)).reshape(B, T, C).astype(np.float32)



# revision 40
# speedup vs baseline: 1.2733x; 1.0023x over previous
"""Cross-level attention (3 KV levels: causal T=2048, full T1=512, full T2=128)
for B=2, H=16, T=2048, DH=64 on 8 Trainium2 NeuronCores.

Sharding: the 32 (b, h) pairs are split 4-per-core (batch + head parallel);
each core computes full attention for its 4 heads, level_w replicated.

Per-core dataflow (all operands resident in SBUF after one initial load):
  - Heads are processed as 2 "duos" (2 pairs packed on partitions 0-63 /
    64-127); per (duo, 512-query block, 128-key block j, half) one QK^T
    matmul produces an S^T tile [s=128, t=512] in a single PSUM bank.
  - exp runs SPLIT ACROSS TWO ENGINES: ACT computes exact exp (scale folded
    in) for the error-sensitive blocks (qb=0, L2, L0 diagonals, and a 1/5
    keep-share); DVE computes a Schraudolph approximate exp for the rest --
    one tensor_scalar affine (s*SCH_A + SCH_B) written as int16 = the bf16
    BIT PATTERN of exp(s/8) (sigma ~1.7% multiplicative noise). Offloaded
    blocks are chosen where softmax averages over >=512 keys, so measured
    output rel-l2 only rises 2.8e-3 -> 4.8e-3 (gate 2e-2).
  - 5 one-bank score buffers circulate: the serial loop exp(k) -> free ->
    QK(k+2) -> exp(k+2) limits exp throughput to bufs/roundtrip; 5 bufs x
    ~1.3us roundtrip sustains the needed ~1 exp/350ns across both engines.
  - Causal masking for level 0 is block-level: fully-masked blocks skipped,
    diagonal blocks get a 0/1 upper-tri multiply post-exp (on Pool).
  - PV: matmul(lhsT=P^T[c-slice], rhs=V'[s-block]) accumulating in 3 PSUM
    banks (one per level), V' carries a ones-column so the denominator
    lands in column 64. Combine = sum_l PV_l * recip(rowsum_l): recip+mul
    on DVE (PSUM-capable), cross-level adds on Pool (SBUF-only engine).
  - The unit pipeline interleaves EMISSION: unit u's PV/combine generator is
    pumped one chunk (PV_CHUNK matmuls) per exp block of unit u+1, so the
    in-order PE always has satisfiable PV work queued ahead of exp-gated QK
    matmuls. The last unit splits halves so its own PV overlaps its phase 1.
  - DMAs execute in issue order, so they are emitted strictly by need time
    (first unit's Q/K slices, duo-0 V, duo-0 remainder, duo-1); V tiles ride
    the Pool SWDGE queue in per-duo transfers.

TimelineSim cost model: ~102.1 us/core (DVE 85.5 busy / ACT 81.6 / PE 71.6 /
Pool 48.3; ~6us DMA-gated startup + ~4us drain tail). CoreSim-validated:
rel-l2 4.7e-3. Down from 130 us for the single-exp-engine ancestor.

Bottleneck notes: exp element throughput is now jointly ACT+DVE-bound
(~167us of exp+combine engine work over two engines); PE sits at 71.6us.
Pool CANNOT read PSUM (BIR verifier) so it can't take exp or combine-mul
work, and dma_start cannot read PSUM either (no staging path) -- Pool is
capped at tri masks + combine adds + V DMAs. Measured dead ends: 2-bank
score tiles with 2 bufs (buffer roundtrip caps exp rate at ~1/950ns),
fp8/DoubleRow QK or PV (quantization alone costs ~2% rel-l2), consolidating
all V DMAs into one transfer (3us serial-DMA blob delays first QK by 2.5us),
and merged 2-bank ACT exp calls + level-sequential PV in 2 pvp banks (cuts
ACT busy 81.6->70.7 and balances engines at ~80, but each level's PV-start
then waits on the busy DVE's combine-mul to free the shared accumulator
bank -- critical path grows, net +4-5us; would need combine prioritized
ahead of queued DVE exps to pay off).
"""
import numpy as np
import ml_dtypes

B, H, T, DH = 2, 16, 2048, 64
C = H * DH
T1, T2 = 512, 128
NCORES = 8
PAIRS = 4          # (b, h) pairs per core
DUOS = 2           # pairs are packed two-per-SBUF-tile
QB = T // 512      # 512-wide query blocks
NJ = (T // 128, T1 // 128, T2 // 128)

TRACE = False          # set by test.py for profiling runs
LAST_RESULT = None     # BassKernelResults from the most recent run

# Schraudolph approximate-exp constants (DVE offload): bf16 bit pattern of
# exp(s/8) ~= int16(round(s * SCH_A + SCH_B)); bias picked for zero mean
# multiplicative error (sigma ~1.7%), applied only to low-error L0 blocks.
SCH_A = 128 * np.log2(np.e) * (DH ** -0.5)
SCH_B = 128 * (127.0 - 0.0397 / np.log(2))
# Of the half-blocks eligible for DVE offload, keep every DVE_SKIP-th on ACT
DVE_SKIP = 5
# phase2 generator yields to the interleaver every PV_CHUNK matmuls
PV_CHUNK = 5

_NC_CACHE = {}


def _build_nc(w):
    import concourse.bass as bass
    from concourse import bacc
    import concourse.tile as tile
    import concourse.mybir as mybir
    from contextlib import ExitStack

    BF16 = mybir.dt.bfloat16
    F32 = mybir.dt.float32
    I16 = mybir.dt.int16
    Exp = mybir.ActivationFunctionType.Exp
    Mult = mybir.AluOpType.mult
    Add = mybir.AluOpType.add

    nc = bacc.Bacc("TRN2", target_bir_lowering=False)
    qt = nc.dram_tensor("qt", [DUOS, 128, T], BF16, kind="ExternalInput")
    k0t = nc.dram_tensor("k0t", [DUOS, 128, T], BF16, kind="ExternalInput")
    k1t = nc.dram_tensor("k1t", [DUOS, 128, T1], BF16, kind="ExternalInput")
    k2t = nc.dram_tensor("k2t", [DUOS, 128, T2], BF16, kind="ExternalInput")
    v0 = nc.dram_tensor("v0", [128, PAIRS, NJ[0], 65], BF16, kind="ExternalInput")
    v1 = nc.dram_tensor("v1", [128, PAIRS, NJ[1], 65], BF16, kind="ExternalInput")
    v2 = nc.dram_tensor("v2", [128, PAIRS, NJ[2], 65], BF16, kind="ExternalInput")
    tri = nc.dram_tensor("tri", [128, 128], BF16, kind="ExternalInput")
    out = nc.dram_tensor("out", [PAIRS, 128, T // 128, DH], F32, kind="ExternalOutput")

    with tile.TileContext(nc) as tc, ExitStack() as ctx:
        const = ctx.enter_context(tc.tile_pool(name="const", bufs=1))
        qkp = ctx.enter_context(tc.tile_pool(name="qkp", bufs=5, space="PSUM"))
        pvp = ctx.enter_context(tc.tile_pool(name="pvp", bufs=3, space="PSUM"))
        pts = ctx.enter_context(tc.tile_pool(name="pts", bufs=88))
        outp = ctx.enter_context(tc.tile_pool(name="outp", bufs=4))
        small = ctx.enter_context(tc.tile_pool(name="small", bufs=8))

        def load(dram_ap, shape, tag):
            t = const.tile(shape, BF16, tag=tag, name=tag)
            nc.sync.dma_start(out=t, in_=dram_ap)
            return t

        # Big Q/K tensors are DMA'd in 512-column chunks so the first
        # query-block's matmuls can start before the full tensors land.
        def load_chunked(dram_ap, shape, tag, chunk=512):
            t = const.tile(shape, BF16, tag=tag, name=tag)
            for c0 in range(0, shape[1], chunk):
                nc.sync.dma_start(out=t[:, c0:c0 + chunk],
                                  in_=dram_ap[:, c0:c0 + chunk])
            return t

        # Emission order = need order: the first unit is (d=0, qb=0), which
        # needs only qt0/k0t0 column chunk 0 plus k1t/k2t of duo 0. V tiles go
        # on the gpsimd SWDGE queue so they stream in parallel with the
        # HWDGE-queued Q/K chunks.
        sb_tri = load(tri[:], [128, 128], "tri")
        def alloc(shape, tag):
            return const.tile(shape, BF16, tag=tag, name=tag)
        sb_qt = [alloc([128, T], f"qt{d}") for d in range(DUOS)]
        sb_kt = [[alloc([128, T], f"k0t{d}") for d in range(DUOS)],
                 [alloc([128, T1], f"k1t{d}") for d in range(DUOS)],
                 [alloc([128, T2], f"k2t{d}") for d in range(DUOS)]]
        # All pairs' V tiles for one level share a single SBUF tile; loaded
        # in per-duo SWDGE DMAs so no single transfer blocks the serial DMA
        # stream for long, ordered by first use.
        sb_v = [alloc([128, PAIRS, NJ[l], 65], f"v{l}") for l in range(3)]

        def dma_cols(t, dram_ap, c0, c1, engine=None):
            (engine or nc.sync).dma_start(out=t[:, c0:c1], in_=dram_ap[:, c0:c1])

        def v_load(l, d):
            dram = (v0, v1, v2)[l]
            nc.gpsimd.dma_start(out=sb_v[l][:, 2 * d:2 * d + 2],
                                in_=dram[:, 2 * d:2 * d + 2])

        # DMAs execute in issue order, so emit strictly by need time:
        # unit (0,0) slices, duo-0 V tiles, rest of duo 0 (qt high chunks
        # first: unit order is qb=3,2,1), then duo 1.
        dma_cols(sb_kt[0][0], k0t[0], 0, 128)
        dma_cols(sb_qt[0], qt[0], 0, 512)
        dma_cols(sb_kt[1][0], k1t[0], 0, 128)
        dma_cols(sb_kt[0][0], k0t[0], 128, 512)
        dma_cols(sb_kt[1][0], k1t[0], 128, T1)
        nc.sync.dma_start(out=sb_kt[2][0], in_=k2t[0])
        for l in range(3):
            v_load(l, 0)
        for c0 in range(512, T, 512):
            dma_cols(sb_kt[0][0], k0t[0], c0, c0 + 512)
        for c0 in (1536, 1024, 512):
            dma_cols(sb_qt[0], qt[0], c0, c0 + 512)
        # duo 1 (first needed at unit (1,3), mid-kernel)
        for c0 in range(0, T, 512):
            dma_cols(sb_kt[0][1], k0t[1], c0, c0 + 512)
        dma_cols(sb_qt[1], qt[1], 1536, 2048)
        for l in range(3):
            v_load(l, 1)
        for c0 in (1024, 512, 0):
            dma_cols(sb_qt[1], qt[1], c0, c0 + 512)
        nc.sync.dma_start(out=sb_kt[1][1], in_=k1t[1])
        nc.sync.dma_start(out=sb_kt[2][1], in_=k2t[1])

        def unit_jblocks(qb):
            return ([(0, j) for j in range(4 * qb + 4)]
                    + [(1, j) for j in range(NJ[1])]
                    + [(2, j) for j in range(NJ[2])])

        def unit_exp_order(qb):
            """Emission order + exp-engine assignment for one unit.

            L0 full blocks of qb>=1 go to DVE via Schraudolph approx exp
            (softmax weight noise ~1.7% there costs ~0.4% output rel-l2
            because those queries average over >=512 keys); qb=0 units
            offload their L1 blocks instead (keeps DVE busy at startup,
            ~0.3% rel-l2). Everything else stays exact on ACT. The two
            streams are interleaved so both engines ping-pong the 2 PSUM
            score buffers concurrently.
            """
            if qb >= 1:
                elig = [(0, j, h) for j in range(4 * qb) for h in range(2)]
            else:
                elig = [(1, j, h) for j in range(NJ[1]) for h in range(2)]
            skip = DVE_SKIP if qb >= 1 else 3
            dve = [it for i, it in enumerate(elig) if i % skip != skip - 1]
            dset = set(dve)
            act = [(l, j, h) for (l, j) in unit_jblocks(qb) for h in range(2)
                   if (l, j, h) not in dset]
            order = []
            na, nd = len(act), len(dve)
            ia = id_ = 0
            for _ in range(na + nd):
                # Bresenham-style proportional interleave of the two streams
                if id_ < nd and (ia >= na or id_ * na < ia * nd):
                    order.append((dve[id_], "V")); id_ += 1
                else:
                    order.append((act[ia], "A")); ia += 1
            return order

        def emit_block(d, qb, l, j, half, eng, pt_tiles):
                # ---- Phase 1 step: S^T = K^T.T @ Q^T for one (j, half),
                # then exp (ACT exact / DVE Schraudolph) -> P^T. One PSUM
                # bank per tile so 4 score buffers circulate concurrently.
                # Diagonal L0 blocks: columns left of the diagonal 128-col
                # sub-block are fully causal-masked and never read by PV,
                # so compute/exp only the [f0, 512) column range.
                f0 = 128 * (j - 4 * qb) if (l == 0 and j >= 4 * qb) else 0
                sp = qkp.tile([128, 512], F32, tag="qk", name="qk")
                nc.tensor.matmul(
                    out=sp[:, f0:],
                    lhsT=sb_kt[l][d][64 * half:64 * half + 64,
                                     128 * j:128 * j + 128],
                    rhs=sb_qt[d][64 * half:64 * half + 64,
                                 512 * qb + f0:512 * qb + 512],
                    start=True, stop=True,
                )
                pt = pts.tile([128, 512], BF16, tag="pt", name="pt")
                if eng == "V":
                    nc.vector.tensor_scalar(
                        out=pt.bitcast(I16), in0=sp,
                        scalar1=float(SCH_A), scalar2=float(SCH_B),
                        op0=Mult, op1=Add)
                else:
                    nc.scalar.activation(out=pt[:, f0:], in_=sp[:, f0:],
                                         func=Exp, scale=DH ** -0.5)
                if l == 0 and j >= 4 * qb:
                    s = pt[:, f0:f0 + 128]
                    nc.gpsimd.tensor_mul(out=s, in0=s, in1=sb_tri)
                pt_tiles[(l, j, half)] = pt

        def phase2_gen(d, qb, pt_tiles):
                jblocks = unit_jblocks(qb)
                # ---- Phase 2: PV accumulation + combine, per pair half.
                # A generator yielding every few matmuls so the driver can
                # interleave this PE-ready work between the next unit's
                # exp-gated QK blocks (keeps the in-order PE stream busy
                # while the exp engines drain).
                for half in range(2):
                    p = 2 * d + half
                    pvt = [pvp.tile([128, 4, 65], F32, tag="pv", name=f"pv{_l}") for _l in range(3)]
                    cnt = 0
                    for (l, j) in jblocks:
                        pt = pt_tiles[(l, j, half)]
                        # One accumulation group per PSUM bank (= per level):
                        # start zeroes the whole 2KB zero region, so only the
                        # very first matmul into the bank may set start=True.
                        lvl_last_j = (4 * qb + 3) if l == 0 else NJ[l] - 1
                        for c in range(4):
                            if l == 0 and j > 4 * qb + c:
                                continue
                            nc.tensor.matmul(
                                out=pvt[l][:, c, :],
                                lhsT=pt[:, 128 * c:128 * c + 128],
                                rhs=sb_v[l][:, p, j, :],
                                start=(j == 0 and c == 0),
                                stop=(j == lvl_last_j and c == 3),
                            )
                            cnt += 1
                            if cnt >= PV_CHUNK:
                                cnt = 0
                                yield
                    osb = outp.tile([128, 4, DH], F32, tag="osb", name="osb")
                    for l in range(3):
                        rc = small.tile([128, 4, 1], F32, tag="rc", name="rc")
                        nc.vector.reciprocal(out=rc[:, :, 0], in_=pvt[l][:, :, 64])
                        dst = osb if l == 0 else outp.tile([128, 4, DH], F32, tag="tmp", name="tmp")
                        nc.vector.tensor_mul(
                            out=dst, in0=pvt[l][:, :, 0:64],
                            in1=rc.broadcast_to([128, 4, DH]))
                        if l > 0:
                            nc.gpsimd.tensor_add(out=osb, in0=osb, in1=dst)
                    nc.sync.dma_start(out=out[p][:, 4 * qb:4 * qb + 4, :],
                                      in_=osb)
                    yield

        # Software pipeline: unit u's PV/combine stream is emitted interleaved
        # between unit u+1's QK+exp blocks (one generator chunk per block), so
        # the in-order PE always has satisfiable PV work queued ahead of each
        # exp-gated QK matmul. Unit order puts a small unit first (fast start
        # after partial DMA) and a small one last (short PV tail).
        units = [(0, 0), (0, 3), (0, 2), (0, 1), (1, 3), (1, 2), (1, 1), (1, 0)]
        gen = None
        for u in units[:-1]:
            d, qb = u
            tiles = {}
            for ((l, j, half), eng) in unit_exp_order(qb):
                emit_block(d, qb, l, j, half, eng, tiles)
                if gen is not None:
                    next(gen, None)
            if gen is not None:
                for _ in gen:
                    pass
            gen = phase2_gen(d, qb, tiles)
        # Last unit: emit half 0's blocks first, then start its own PV
        # generator during half 1's blocks (safe pumps only reach half-0
        # PV work), trimming the drain tail to half 1's PV + combine.
        d, qb = units[-1]
        tiles = {}
        order = unit_exp_order(qb)
        h0 = [it for it in order if it[0][2] == 0]
        h1 = [it for it in order if it[0][2] == 1]
        for ((l, j, half), eng) in h0:
            emit_block(d, qb, l, j, half, eng, tiles)
            if gen is not None:
                next(gen, None)
        for _ in gen:
            pass
        gen = phase2_gen(d, qb, tiles)
        safe = (16 * qb + 30) // PV_CHUNK + 1
        for i, ((l, j, half), eng) in enumerate(h1):
            emit_block(d, qb, l, j, half, eng, tiles)
            if safe > 0:
                next(gen, None)
                safe -= 1
        for _ in gen:
            pass
    nc.compile()
    return nc


def _prepare(inputs):
    bf = ml_dtypes.bfloat16
    Q = np.asarray(inputs["Q"], np.float32)
    Ks = [np.asarray(inputs[k], np.float32) for k in ("K0", "K1", "K2")]
    Vs = [np.asarray(inputs[k], np.float32) for k in ("V0", "V1", "V2")]
    level_w = np.asarray(inputs["level_w"], np.float64)
    e = np.exp(level_w - level_w.max())
    w = (e / e.sum()).astype(np.float64)

    # Host-side layout for sharding: per-head transposed Q/K ([64, Tm]) and
    # s-tiled V with a ones column ([128, nj, 65]).
    QT = np.ascontiguousarray(Q.transpose(0, 1, 3, 2)).astype(bf)  # [B,H,64,T]
    KTs = []
    for Kl in Ks:
        Tm = Kl.shape[1]
        Kh = Kl.reshape(B, Tm, H, DH).transpose(0, 2, 3, 1)  # [B,H,64,Tm]
        KTs.append(np.ascontiguousarray(Kh).astype(bf))
    Vps = []
    for lvl, Vl in enumerate(Vs):
        Tm = Vl.shape[1]
        Vl = Vl * np.float32(w[lvl])   # fold level weight into V (exact in fp32)
        Vh = Vl.reshape(B, Tm, H, DH).transpose(0, 2, 1, 3)  # [B,H,Tm,64]
        vp = np.ones((B, H, Tm // 128, 128, 65), np.float32)
        vp[..., :64] = Vh.reshape(B, H, Tm // 128, 128, DH)
        # -> [B, H, 128(p), nj, 65]
        Vps.append(np.ascontiguousarray(vp.transpose(0, 1, 3, 2, 4)).astype(bf))
    tri = (np.arange(128)[:, None] <= np.arange(128)[None, :]).astype(bf)

    in_maps = []
    for core in range(NCORES):
        m = {
            "qt": np.empty((DUOS, 128, T), bf),
            "k0t": np.empty((DUOS, 128, T), bf),
            "k1t": np.empty((DUOS, 128, T1), bf),
            "k2t": np.empty((DUOS, 128, T2), bf),
            "v0": np.empty((128, PAIRS, NJ[0], 65), bf),
            "v1": np.empty((128, PAIRS, NJ[1], 65), bf),
            "v2": np.empty((128, PAIRS, NJ[2], 65), bf),
            "tri": tri,
        }
        for p in range(PAIRS):
            g = PAIRS * core + p
            b, h = divmod(g, H)
            d, half = divmod(p, 2)
            sl = slice(64 * half, 64 * half + 64)
            m["qt"][d, sl] = QT[b, h]
            m["k0t"][d, sl] = KTs[0][b, h]
            m["k1t"][d, sl] = KTs[1][b, h]
            m["k2t"][d, sl] = KTs[2][b, h]
            m["v0"][:, p] = Vps[0][b, h]
            m["v1"][:, p] = Vps[1][b, h]
            m["v2"][:, p] = Vps[2][b, h]
        in_maps.append(m)

    return in_maps, w


def kernel(**inputs):
    global LAST_RESULT
    from concourse.bass_utils import run_bass_kernel_spmd

    in_maps, w = _prepare(inputs)
    key = tuple(np.asarray(w, np.float64).tolist())
    if key not in _NC_CACHE:
        _NC_CACHE[key] = _build_nc(w)
    nc = _NC_CACHE[key]

    try:
        res = run_bass_kernel_spmd(nc, in_maps, core_ids=list(range(NCORES)),
                                   trace=TRACE)
    except (ImportError, ModuleNotFoundError):
        # axon build without the NTFF profiling hook — run without trace
        res = run_bass_kernel_spmd(nc, in_maps, core_ids=list(range(NCORES)),
                                   trace=False)
    LAST_RESULT = res

    outs = np.stack([np.asarray(r["out"]) for r in res.results])  # [8,4,128,16,64]
    O = outs.transpose(0, 1, 3, 2, 4).reshape(B, H, T, DH)  # t = 128*n + pp
    return np.ascontiguousarray(O.transpose(0, 2, 1, 3)).reshape(B, T, C).astype(np.float32)



# revision 44
# speedup vs baseline: 1.2764x; 1.0024x over previous
"""Cross-level attention (3 KV levels: causal T=2048, full T1=512, full T2=128)
for B=2, H=16, T=2048, DH=64 on 8 Trainium2 NeuronCores.

Sharding: the 32 (b, h) pairs are split 4-per-core (batch + head parallel);
each core computes full attention for its 4 heads, level_w replicated.

Per-core dataflow (all operands resident in SBUF after one initial load):
  - Heads are processed as 2 "duos" (2 pairs packed on partitions 0-63 /
    64-127); per (duo, 512-query block, 128-key block j, half) one QK^T
    matmul produces an S^T tile [s=128, t=512] in a single PSUM bank.
  - exp runs SPLIT ACROSS TWO ENGINES: ACT computes exact exp (scale folded
    in) for the error-sensitive blocks (qb=0, L2, L0 diagonals, and a 1/5
    keep-share); DVE computes a Schraudolph approximate exp for the rest --
    one tensor_scalar affine (s*SCH_A + SCH_B) written as int16 = the bf16
    BIT PATTERN of exp(s/8) (sigma ~1.7% multiplicative noise). Offloaded
    blocks are chosen where softmax averages over >=512 keys, so measured
    output rel-l2 only rises 2.8e-3 -> 4.8e-3 (gate 2e-2).
  - 5 one-bank score buffers circulate: the serial loop exp(k) -> free ->
    QK(k+2) -> exp(k+2) limits exp throughput to bufs/roundtrip; 5 bufs x
    ~1.3us roundtrip sustains the needed ~1 exp/350ns across both engines.
  - Causal masking for level 0 is block-level: fully-masked blocks skipped,
    diagonal blocks get a 0/1 upper-tri multiply post-exp (on Pool).
  - PV: matmul(lhsT=P^T[c-slice], rhs=V'[s-block]) accumulating in 3 PSUM
    banks (one per level), V' carries a ones-column so the denominator
    lands in column 64. Combine = sum_l PV_l * recip(rowsum_l): recip+mul
    on DVE (PSUM-capable), cross-level adds on Pool (SBUF-only engine).
  - The unit pipeline interleaves EMISSION: unit u's PV/combine generator is
    pumped one chunk (PV_CHUNK matmuls) per exp block of unit u+1, so the
    in-order PE always has satisfiable PV work queued ahead of exp-gated QK
    matmuls. The last unit splits halves so its own PV overlaps its phase 1.
  - DMAs execute in issue order, so they are emitted strictly by need time
    (first unit's Q/K slices, duo-0 V, duo-0 remainder, duo-1); V tiles ride
    the Pool SWDGE queue in per-duo transfers.

TimelineSim cost model: ~101.9 us/core (ACT 84.1 busy / DVE 82.9 / PE 71.4 /
Pool 48.3; ~6us DMA-gated startup + ~4us drain tail). CoreSim-validated:
rel-l2 4.7e-3. Down from 130 us for the single-exp-engine ancestor.

Bottleneck notes: exp element throughput is now jointly ACT+DVE-bound
(~167us of exp+combine engine work over two engines); PE sits at 71.6us.
Pool CANNOT read PSUM (BIR verifier) so it can't take exp or combine-mul
work, and dma_start cannot read PSUM either (no staging path) -- Pool is
capped at tri masks + combine adds + V DMAs. Measured dead ends: 2-bank
score tiles with 2 bufs (buffer roundtrip caps exp rate at ~1/950ns),
fp8/DoubleRow QK or PV (quantization alone costs ~2% rel-l2), consolidating
all V DMAs into one transfer (3us serial-DMA blob delays first QK by 2.5us),
and merged 2-bank ACT exp calls + level-sequential PV in 2 pvp banks (cuts
ACT busy 81.6->70.7 and balances engines at ~80, but each level's PV-start
then waits on the busy DVE's combine-mul to free the shared accumulator
bank -- critical path grows, net +4-5us; would need combine prioritized
ahead of queued DVE exps to pay off).
"""
import numpy as np
import ml_dtypes

B, H, T, DH = 2, 16, 2048, 64
C = H * DH
T1, T2 = 512, 128
NCORES = 8
PAIRS = 4          # (b, h) pairs per core
DUOS = 2           # pairs are packed two-per-SBUF-tile
QB = T // 512      # 512-wide query blocks
NJ = (T // 128, T1 // 128, T2 // 128)

TRACE = False          # set by test.py for profiling runs
LAST_RESULT = None     # BassKernelResults from the most recent run

# Schraudolph approximate-exp constants (DVE offload): bf16 bit pattern of
# exp(s/8) ~= int16(round(s * SCH_A + SCH_B)); bias picked for zero mean
# multiplicative error (sigma ~1.7%), applied only to low-error L0 blocks.
SCH_A = 128 * np.log2(np.e) * (DH ** -0.5)
SCH_B = 128 * (127.0 - 0.0397 / np.log(2))
# Of the half-blocks eligible for DVE offload, keep every DVE_SKIP-th on ACT
DVE_SKIP = 5
# phase2 generator yields to the interleaver every PV_CHUNK matmuls
PV_CHUNK = 5

_NC_CACHE = {}


def _build_nc(w):
    import concourse.bass as bass
    from concourse import bacc
    import concourse.tile as tile
    import concourse.mybir as mybir
    from contextlib import ExitStack

    BF16 = mybir.dt.bfloat16
    F32 = mybir.dt.float32
    I16 = mybir.dt.int16
    Exp = mybir.ActivationFunctionType.Exp
    Mult = mybir.AluOpType.mult
    Add = mybir.AluOpType.add

    nc = bacc.Bacc("TRN2", target_bir_lowering=False)
    qt = nc.dram_tensor("qt", [DUOS, 128, T], BF16, kind="ExternalInput")
    k0t = nc.dram_tensor("k0t", [DUOS, 128, T], BF16, kind="ExternalInput")
    k1t = nc.dram_tensor("k1t", [DUOS, 128, T1], BF16, kind="ExternalInput")
    k2t = nc.dram_tensor("k2t", [DUOS, 128, T2], BF16, kind="ExternalInput")
    v0 = nc.dram_tensor("v0", [128, PAIRS, NJ[0], 65], BF16, kind="ExternalInput")
    v1 = nc.dram_tensor("v1", [128, PAIRS, NJ[1], 65], BF16, kind="ExternalInput")
    v2 = nc.dram_tensor("v2", [128, PAIRS, NJ[2], 65], BF16, kind="ExternalInput")
    tri = nc.dram_tensor("tri", [128, 128], BF16, kind="ExternalInput")
    out = nc.dram_tensor("out", [PAIRS, 128, T // 128, DH], F32, kind="ExternalOutput")

    with tile.TileContext(nc) as tc, ExitStack() as ctx:
        const = ctx.enter_context(tc.tile_pool(name="const", bufs=1))
        qkp = ctx.enter_context(tc.tile_pool(name="qkp", bufs=5, space="PSUM"))
        pvp = ctx.enter_context(tc.tile_pool(name="pvp", bufs=3, space="PSUM"))
        pts = ctx.enter_context(tc.tile_pool(name="pts", bufs=88))
        outp = ctx.enter_context(tc.tile_pool(name="outp", bufs=4))
        small = ctx.enter_context(tc.tile_pool(name="small", bufs=8))

        def load(dram_ap, shape, tag):
            t = const.tile(shape, BF16, tag=tag, name=tag)
            nc.sync.dma_start(out=t, in_=dram_ap)
            return t

        # Big Q/K tensors are DMA'd in 512-column chunks so the first
        # query-block's matmuls can start before the full tensors land.
        def load_chunked(dram_ap, shape, tag, chunk=512):
            t = const.tile(shape, BF16, tag=tag, name=tag)
            for c0 in range(0, shape[1], chunk):
                nc.sync.dma_start(out=t[:, c0:c0 + chunk],
                                  in_=dram_ap[:, c0:c0 + chunk])
            return t

        # Emission order = need order: the first unit is (d=0, qb=0), which
        # needs only qt0/k0t0 column chunk 0 plus k1t/k2t of duo 0. V tiles go
        # on the gpsimd SWDGE queue so they stream in parallel with the
        # HWDGE-queued Q/K chunks.
        sb_tri = load(tri[:], [128, 128], "tri")
        def alloc(shape, tag):
            return const.tile(shape, BF16, tag=tag, name=tag)
        sb_qt = [alloc([128, T], f"qt{d}") for d in range(DUOS)]
        sb_kt = [[alloc([128, T], f"k0t{d}") for d in range(DUOS)],
                 [alloc([128, T1], f"k1t{d}") for d in range(DUOS)],
                 [alloc([128, T2], f"k2t{d}") for d in range(DUOS)]]
        # All pairs' V tiles for one level share a single SBUF tile; loaded
        # in per-duo SWDGE DMAs so no single transfer blocks the serial DMA
        # stream for long, ordered by first use.
        sb_v = [alloc([128, PAIRS, NJ[l], 65], f"v{l}") for l in range(3)]

        def dma_cols(t, dram_ap, c0, c1, engine=None):
            (engine or nc.sync).dma_start(out=t[:, c0:c1], in_=dram_ap[:, c0:c1])

        def v_load(l, d):
            dram = (v0, v1, v2)[l]
            nc.gpsimd.dma_start(out=sb_v[l][:, 2 * d:2 * d + 2],
                                in_=dram[:, 2 * d:2 * d + 2])

        # DMAs execute in issue order, so emit strictly by need time:
        # unit (0,0) slices, duo-0 V tiles, rest of duo 0 (qt high chunks
        # first: unit order is qb=3,2,1), then duo 1.
        dma_cols(sb_kt[0][0], k0t[0], 0, 128)
        dma_cols(sb_qt[0], qt[0], 0, 512)
        dma_cols(sb_kt[1][0], k1t[0], 0, 128)
        dma_cols(sb_kt[0][0], k0t[0], 128, 512)
        dma_cols(sb_kt[1][0], k1t[0], 128, T1)
        nc.sync.dma_start(out=sb_kt[2][0], in_=k2t[0])
        for l in range(3):
            v_load(l, 0)
        for c0 in range(512, T, 512):
            dma_cols(sb_kt[0][0], k0t[0], c0, c0 + 512)
        for c0 in (1536, 1024, 512):
            dma_cols(sb_qt[0], qt[0], c0, c0 + 512)
        # duo 1 (first needed at unit (1,3), mid-kernel)
        for c0 in range(0, T, 512):
            dma_cols(sb_kt[0][1], k0t[1], c0, c0 + 512)
        dma_cols(sb_qt[1], qt[1], 1536, 2048)
        for l in range(3):
            v_load(l, 1)
        for c0 in (1024, 512, 0):
            dma_cols(sb_qt[1], qt[1], c0, c0 + 512)
        nc.sync.dma_start(out=sb_kt[1][1], in_=k1t[1])
        nc.sync.dma_start(out=sb_kt[2][1], in_=k2t[1])

        def unit_jblocks(qb):
            return ([(0, j) for j in range(4 * qb + 4)]
                    + [(1, j) for j in range(NJ[1])]
                    + [(2, j) for j in range(NJ[2])])

        def unit_exp_order(qb):
            """Emission order + exp-engine assignment for one unit.

            L0 full blocks of qb>=1 go to DVE via Schraudolph approx exp
            (softmax weight noise ~1.7% there costs ~0.4% output rel-l2
            because those queries average over >=512 keys); qb=0 units
            offload their L1 blocks instead (keeps DVE busy at startup,
            ~0.3% rel-l2). Everything else stays exact on ACT. The two
            streams are interleaved so both engines ping-pong the 2 PSUM
            score buffers concurrently.
            """
            if qb >= 1:
                elig = [(0, j, h) for j in range(4 * qb) for h in range(2)]
            else:
                elig = [(1, j, h) for j in range(NJ[1]) for h in range(2)]
            skip = DVE_SKIP if qb >= 1 else 2
            dve = [it for i, it in enumerate(elig) if i % skip != skip - 1]
            dset = set(dve)
            act = [(l, j, h) for (l, j) in unit_jblocks(qb) for h in range(2)
                   if (l, j, h) not in dset]
            order = []
            na, nd = len(act), len(dve)
            ia = id_ = 0
            for _ in range(na + nd):
                # Bresenham-style proportional interleave of the two streams
                if id_ < nd and (ia >= na or id_ * na < ia * nd):
                    order.append((dve[id_], "V")); id_ += 1
                else:
                    order.append((act[ia], "A")); ia += 1
            return order

        def emit_block(d, qb, l, j, half, eng, pt_tiles):
                # ---- Phase 1 step: S^T = K^T.T @ Q^T for one (j, half),
                # then exp (ACT exact / DVE Schraudolph) -> P^T. One PSUM
                # bank per tile so 4 score buffers circulate concurrently.
                # Diagonal L0 blocks: columns left of the diagonal 128-col
                # sub-block are fully causal-masked and never read by PV,
                # so compute/exp only the [f0, 512) column range.
                f0 = 128 * (j - 4 * qb) if (l == 0 and j >= 4 * qb) else 0
                sp = qkp.tile([128, 512], F32, tag="qk", name="qk")
                nc.tensor.matmul(
                    out=sp[:, f0:],
                    lhsT=sb_kt[l][d][64 * half:64 * half + 64,
                                     128 * j:128 * j + 128],
                    rhs=sb_qt[d][64 * half:64 * half + 64,
                                 512 * qb + f0:512 * qb + 512],
                    start=True, stop=True,
                )
                pt = pts.tile([128, 512], BF16, tag="pt", name="pt")
                if eng == "V":
                    nc.vector.tensor_scalar(
                        out=pt.bitcast(I16), in0=sp,
                        scalar1=float(SCH_A), scalar2=float(SCH_B),
                        op0=Mult, op1=Add)
                else:
                    nc.scalar.activation(out=pt[:, f0:], in_=sp[:, f0:],
                                         func=Exp, scale=DH ** -0.5)
                if l == 0 and j >= 4 * qb:
                    s = pt[:, f0:f0 + 128]
                    nc.gpsimd.tensor_mul(out=s, in0=s, in1=sb_tri)
                pt_tiles[(l, j, half)] = pt

        def phase2_gen(d, qb, pt_tiles):
                jblocks = unit_jblocks(qb)
                # ---- Phase 2: PV accumulation + combine, per pair half.
                # A generator yielding every few matmuls so the driver can
                # interleave this PE-ready work between the next unit's
                # exp-gated QK blocks (keeps the in-order PE stream busy
                # while the exp engines drain).
                for half in range(2):
                    p = 2 * d + half
                    pvt = [pvp.tile([128, 4, 65], F32, tag="pv", name=f"pv{_l}") for _l in range(3)]
                    cnt = 0
                    for (l, j) in jblocks:
                        pt = pt_tiles[(l, j, half)]
                        # One accumulation group per PSUM bank (= per level):
                        # start zeroes the whole 2KB zero region, so only the
                        # very first matmul into the bank may set start=True.
                        lvl_last_j = (4 * qb + 3) if l == 0 else NJ[l] - 1
                        for c in range(4):
                            if l == 0 and j > 4 * qb + c:
                                continue
                            nc.tensor.matmul(
                                out=pvt[l][:, c, :],
                                lhsT=pt[:, 128 * c:128 * c + 128],
                                rhs=sb_v[l][:, p, j, :],
                                start=(j == 0 and c == 0),
                                stop=(j == lvl_last_j and c == 3),
                            )
                            cnt += 1
                            if cnt >= PV_CHUNK:
                                cnt = 0
                                yield
                    osb = outp.tile([128, 4, DH], F32, tag="osb", name="osb")
                    for l in range(3):
                        rc = small.tile([128, 4, 1], F32, tag="rc", name="rc")
                        nc.vector.reciprocal(out=rc[:, :, 0], in_=pvt[l][:, :, 64])
                        dst = osb if l == 0 else outp.tile([128, 4, DH], F32, tag="tmp", name="tmp")
                        nc.vector.tensor_mul(
                            out=dst, in0=pvt[l][:, :, 0:64],
                            in1=rc.broadcast_to([128, 4, DH]))
                        if l > 0:
                            nc.gpsimd.tensor_add(out=osb, in0=osb, in1=dst)
                    nc.sync.dma_start(out=out[p][:, 4 * qb:4 * qb + 4, :],
                                      in_=osb)
                    yield

        # Software pipeline: unit u's PV/combine stream is emitted interleaved
        # between unit u+1's QK+exp blocks (one generator chunk per block), so
        # the in-order PE always has satisfiable PV work queued ahead of each
        # exp-gated QK matmul. Unit order puts a small unit first (fast start
        # after partial DMA) and a small one last (short PV tail).
        units = [(0, 0), (0, 3), (0, 2), (0, 1), (1, 3), (1, 2), (1, 1), (1, 0)]
        gen = None
        for u in units[:-1]:
            d, qb = u
            tiles = {}
            for ((l, j, half), eng) in unit_exp_order(qb):
                emit_block(d, qb, l, j, half, eng, tiles)
                if gen is not None:
                    next(gen, None)
            if gen is not None:
                for _ in gen:
                    pass
            gen = phase2_gen(d, qb, tiles)
        # Last unit: emit half 0's blocks first, then start its own PV
        # generator during half 1's blocks (safe pumps only reach half-0
        # PV work), trimming the drain tail to half 1's PV + combine.
        d, qb = units[-1]
        tiles = {}
        order = unit_exp_order(qb)
        h0 = [it for it in order if it[0][2] == 0]
        h1 = [it for it in order if it[0][2] == 1]
        for ((l, j, half), eng) in h0:
            emit_block(d, qb, l, j, half, eng, tiles)
            if gen is not None:
                next(gen, None)
        for _ in gen:
            pass
        gen = phase2_gen(d, qb, tiles)
        safe = (16 * qb + 30) // PV_CHUNK + 1
        for i, ((l, j, half), eng) in enumerate(h1):
            emit_block(d, qb, l, j, half, eng, tiles)
            if safe > 0:
                next(gen, None)
                safe -= 1
        for _ in gen:
            pass
    nc.compile()
    return nc


def _prepare(inputs):
    bf = ml_dtypes.bfloat16
    Q = np.asarray(inputs["Q"], np.float32)
    Ks = [np.asarray(inputs[k], np.float32) for k in ("K0", "K1", "K2")]
    Vs = [np.asarray(inputs[k], np.float32) for k in ("V0", "V1", "V2")]
    level_w = np.asarray(inputs["level_w"], np.float64)
    e = np.exp(level_w - level_w.max())
    w = (e / e.sum()).astype(np.float64)

    # Host-side layout for sharding: per-head transposed Q/K ([64, Tm]) and
    # s-tiled V with a ones column ([128, nj, 65]).
    QT = np.ascontiguousarray(Q.transpose(0, 1, 3, 2)).astype(bf)  # [B,H,64,T]
    KTs = []
    for Kl in Ks:
        Tm = Kl.shape[1]
        Kh = Kl.reshape(B, Tm, H, DH).transpose(0, 2, 3, 1)  # [B,H,64,Tm]
        KTs.append(np.ascontiguousarray(Kh).astype(bf))
    Vps = []
    for lvl, Vl in enumerate(Vs):
        Tm = Vl.shape[1]
        Vl = Vl * np.float32(w[lvl])   # fold level weight into V (exact in fp32)
        Vh = Vl.reshape(B, Tm, H, DH).transpose(0, 2, 1, 3)  # [B,H,Tm,64]
        vp = np.ones((B, H, Tm // 128, 128, 65), np.float32)
        vp[..., :64] = Vh.reshape(B, H, Tm // 128, 128, DH)
        # -> [B, H, 128(p), nj, 65]
        Vps.append(np.ascontiguousarray(vp.transpose(0, 1, 3, 2, 4)).astype(bf))
    tri = (np.arange(128)[:, None] <= np.arange(128)[None, :]).astype(bf)

    in_maps = []
    for core in range(NCORES):
        m = {
            "qt": np.empty((DUOS, 128, T), bf),
            "k0t": np.empty((DUOS, 128, T), bf),
            "k1t": np.empty((DUOS, 128, T1), bf),
            "k2t": np.empty((DUOS, 128, T2), bf),
            "v0": np.empty((128, PAIRS, NJ[0], 65), bf),
            "v1": np.empty((128, PAIRS, NJ[1], 65), bf),
            "v2": np.empty((128, PAIRS, NJ[2], 65), bf),
            "tri": tri,
        }
        for p in range(PAIRS):
            g = PAIRS * core + p
            b, h = divmod(g, H)
            d, half = divmod(p, 2)
            sl = slice(64 * half, 64 * half + 64)
            m["qt"][d, sl] = QT[b, h]
            m["k0t"][d, sl] = KTs[0][b, h]
            m["k1t"][d, sl] = KTs[1][b, h]
            m["k2t"][d, sl] = KTs[2][b, h]
            m["v0"][:, p] = Vps[0][b, h]
            m["v1"][:, p] = Vps[1][b, h]
            m["v2"][:, p] = Vps[2][b, h]
        in_maps.append(m)

    return in_maps, w


def kernel(**inputs):
    global LAST_RESULT
    from concourse.bass_utils import run_bass_kernel_spmd

    in_maps, w = _prepare(inputs)
    key = tuple(np.asarray(w, np.float64).tolist())
    if key not in _NC_CACHE:
        _NC_CACHE[key] = _build_nc(w)
    nc = _NC_CACHE[key]

    try:
        res = run_bass_kernel_spmd(nc, in_maps, core_ids=list(range(NCORES)),
                                   trace=TRACE)
    except (ImportError, ModuleNotFoundError):
        # axon build without the NTFF profiling hook — run without trace
        res = run_bass_kernel_spmd(nc, in_maps, core_ids=list(range(NCORES)),
                                   trace=False)
    LAST_RESULT = res

    outs = np.stack([np.asarray(r["out"]) for r in res.results])  # [8,4,128,16,64]
    O = outs.transpose(0, 1, 3, 2, 4).reshape(B, H, T, DH)  # t = 128*n + pp
    return np.ascontiguousarray(O.transpose(0, 2, 1, 3)).reshape(B, T, C).astype(np.float32)



# revision 46
# speedup vs baseline: 1.2792x; 1.0022x over previous
"""Cross-level attention (3 KV levels: causal T=2048, full T1=512, full T2=128)
for B=2, H=16, T=2048, DH=64 on 8 Trainium2 NeuronCores.

Sharding: the 32 (b, h) pairs are split 4-per-core (batch + head parallel);
each core computes full attention for its 4 heads, level_w replicated.

Per-core dataflow (all operands resident in SBUF after one initial load):
  - Heads are processed as 2 "duos" (2 pairs packed on partitions 0-63 /
    64-127); per (duo, 512-query block, 128-key block j, half) one QK^T
    matmul produces an S^T tile [s=128, t=512] in a single PSUM bank.
  - exp runs SPLIT ACROSS TWO ENGINES: ACT computes exact exp (scale folded
    in) for the error-sensitive blocks (qb=0, L2, L0 diagonals, and a 1/5
    keep-share); DVE computes a Schraudolph approximate exp for the rest --
    one tensor_scalar affine (s*SCH_A + SCH_B) written as int16 = the bf16
    BIT PATTERN of exp(s/8) (sigma ~1.7% multiplicative noise). Offloaded
    blocks are chosen where softmax averages over >=512 keys, so measured
    output rel-l2 only rises 2.8e-3 -> 4.8e-3 (gate 2e-2).
  - 5 one-bank score buffers circulate: the serial loop exp(k) -> free ->
    QK(k+2) -> exp(k+2) limits exp throughput to bufs/roundtrip; 5 bufs x
    ~1.3us roundtrip sustains the needed ~1 exp/350ns across both engines.
  - Causal masking for level 0 is block-level: fully-masked blocks skipped,
    diagonal blocks get a 0/1 upper-tri multiply post-exp (on Pool).
  - PV: matmul(lhsT=P^T[c-slice], rhs=V'[s-block]) accumulating in 3 PSUM
    banks (one per level), V' carries a ones-column so the denominator
    lands in column 64. Combine = sum_l PV_l * recip(rowsum_l): recip+mul
    on DVE (PSUM-capable), cross-level adds on Pool (SBUF-only engine).
  - The unit pipeline interleaves EMISSION: unit u's PV/combine generator is
    pumped one chunk (PV_CHUNK matmuls) per exp block of unit u+1, so the
    in-order PE always has satisfiable PV work queued ahead of exp-gated QK
    matmuls. The last unit splits halves so its own PV overlaps its phase 1.
  - DMAs execute in issue order, so they are emitted strictly by need time
    (first unit's Q/K slices, duo-0 V, duo-0 remainder, duo-1); V tiles ride
    the Pool SWDGE queue in per-duo transfers.

TimelineSim cost model: ~101.6 us/core (ACT 84.1 busy / DVE 82.9 / PE 71.4 /
Pool 48.3; ~6us DMA-gated startup + ~4us drain tail). CoreSim-validated:
rel-l2 4.7e-3. Down from 130 us for the single-exp-engine ancestor.

Bottleneck notes: exp element throughput is now jointly ACT+DVE-bound
(~167us of exp+combine engine work over two engines); PE sits at 71.6us.
Pool CANNOT read PSUM (BIR verifier) so it can't take exp or combine-mul
work, and dma_start cannot read PSUM either (no staging path) -- Pool is
capped at tri masks + combine adds + V DMAs. Measured dead ends: 2-bank
score tiles with 2 bufs (buffer roundtrip caps exp rate at ~1/950ns),
fp8/DoubleRow QK or PV (quantization alone costs ~2% rel-l2), consolidating
all V DMAs into one transfer (3us serial-DMA blob delays first QK by 2.5us),
and merged 2-bank ACT exp calls + level-sequential PV in 2 pvp banks (cuts
ACT busy 81.6->70.7 and balances engines at ~80, but each level's PV-start
then waits on the busy DVE's combine-mul to free the shared accumulator
bank -- critical path grows, net +4-5us; would need combine prioritized
ahead of queued DVE exps to pay off).
"""
import numpy as np
import ml_dtypes

B, H, T, DH = 2, 16, 2048, 64
C = H * DH
T1, T2 = 512, 128
NCORES = 8
PAIRS = 4          # (b, h) pairs per core
DUOS = 2           # pairs are packed two-per-SBUF-tile
QB = T // 512      # 512-wide query blocks
NJ = (T // 128, T1 // 128, T2 // 128)

TRACE = False          # set by test.py for profiling runs
LAST_RESULT = None     # BassKernelResults from the most recent run

# Schraudolph approximate-exp constants (DVE offload): bf16 bit pattern of
# exp(s/8) ~= int16(round(s * SCH_A + SCH_B)); bias picked for zero mean
# multiplicative error (sigma ~1.7%), applied only to low-error L0 blocks.
SCH_A = 128 * np.log2(np.e) * (DH ** -0.5)
SCH_B = 128 * (127.0 - 0.0397 / np.log(2))
# Of the half-blocks eligible for DVE offload, keep every DVE_SKIP-th on ACT
DVE_SKIP = 5
# phase2 generator yields to the interleaver every PV_CHUNK matmuls
PV_CHUNK = 5

_NC_CACHE = {}


def _build_nc(w):
    import concourse.bass as bass
    from concourse import bacc
    import concourse.tile as tile
    import concourse.mybir as mybir
    from contextlib import ExitStack

    BF16 = mybir.dt.bfloat16
    F32 = mybir.dt.float32
    I16 = mybir.dt.int16
    Exp = mybir.ActivationFunctionType.Exp
    Mult = mybir.AluOpType.mult
    Add = mybir.AluOpType.add

    nc = bacc.Bacc("TRN2", target_bir_lowering=False)
    qt = nc.dram_tensor("qt", [DUOS, 128, T], BF16, kind="ExternalInput")
    k0t = nc.dram_tensor("k0t", [DUOS, 128, T], BF16, kind="ExternalInput")
    k1t = nc.dram_tensor("k1t", [DUOS, 128, T1], BF16, kind="ExternalInput")
    k2t = nc.dram_tensor("k2t", [DUOS, 128, T2], BF16, kind="ExternalInput")
    v0 = nc.dram_tensor("v0", [128, PAIRS, NJ[0], 65], BF16, kind="ExternalInput")
    v1 = nc.dram_tensor("v1", [128, PAIRS, NJ[1], 65], BF16, kind="ExternalInput")
    v2 = nc.dram_tensor("v2", [128, PAIRS, NJ[2], 65], BF16, kind="ExternalInput")
    tri = nc.dram_tensor("tri", [128, 128], BF16, kind="ExternalInput")
    out = nc.dram_tensor("out", [PAIRS, 128, T // 128, DH], F32, kind="ExternalOutput")

    with tile.TileContext(nc) as tc, ExitStack() as ctx:
        const = ctx.enter_context(tc.tile_pool(name="const", bufs=1))
        qkp = ctx.enter_context(tc.tile_pool(name="qkp", bufs=5, space="PSUM"))
        pvp = ctx.enter_context(tc.tile_pool(name="pvp", bufs=3, space="PSUM"))
        pts = ctx.enter_context(tc.tile_pool(name="pts", bufs=88))
        outp = ctx.enter_context(tc.tile_pool(name="outp", bufs=4))
        small = ctx.enter_context(tc.tile_pool(name="small", bufs=8))

        def load(dram_ap, shape, tag):
            t = const.tile(shape, BF16, tag=tag, name=tag)
            nc.sync.dma_start(out=t, in_=dram_ap)
            return t

        # Big Q/K tensors are DMA'd in 512-column chunks so the first
        # query-block's matmuls can start before the full tensors land.
        def load_chunked(dram_ap, shape, tag, chunk=512):
            t = const.tile(shape, BF16, tag=tag, name=tag)
            for c0 in range(0, shape[1], chunk):
                nc.sync.dma_start(out=t[:, c0:c0 + chunk],
                                  in_=dram_ap[:, c0:c0 + chunk])
            return t

        # Emission order = need order: the first unit is (d=0, qb=0), which
        # needs only qt0/k0t0 column chunk 0 plus k1t/k2t of duo 0. V tiles go
        # on the gpsimd SWDGE queue so they stream in parallel with the
        # HWDGE-queued Q/K chunks.
        sb_tri = const.tile([128, 128], BF16, tag="tri", name="tri")
        nc.gpsimd.dma_start(out=sb_tri, in_=tri[:])
        def alloc(shape, tag):
            return const.tile(shape, BF16, tag=tag, name=tag)
        sb_qt = [alloc([128, T], f"qt{d}") for d in range(DUOS)]
        sb_kt = [[alloc([128, T], f"k0t{d}") for d in range(DUOS)],
                 [alloc([128, T1], f"k1t{d}") for d in range(DUOS)],
                 [alloc([128, T2], f"k2t{d}") for d in range(DUOS)]]
        # All pairs' V tiles for one level share a single SBUF tile; loaded
        # in per-duo SWDGE DMAs so no single transfer blocks the serial DMA
        # stream for long, ordered by first use.
        sb_v = [alloc([128, PAIRS, NJ[l], 65], f"v{l}") for l in range(3)]

        def dma_cols(t, dram_ap, c0, c1, engine=None):
            (engine or nc.sync).dma_start(out=t[:, c0:c1], in_=dram_ap[:, c0:c1])

        def v_load(l, d):
            dram = (v0, v1, v2)[l]
            nc.gpsimd.dma_start(out=sb_v[l][:, 2 * d:2 * d + 2],
                                in_=dram[:, 2 * d:2 * d + 2])

        # DMAs execute in issue order, so emit strictly by need time:
        # unit (0,0) slices, duo-0 V tiles, rest of duo 0 (qt high chunks
        # first: unit order is qb=3,2,1), then duo 1.
        dma_cols(sb_kt[0][0], k0t[0], 0, 128)
        dma_cols(sb_kt[1][0], k1t[0], 0, 128)
        dma_cols(sb_qt[0], qt[0], 0, 512)
        dma_cols(sb_kt[0][0], k0t[0], 128, 512)
        dma_cols(sb_kt[1][0], k1t[0], 128, T1)
        nc.sync.dma_start(out=sb_kt[2][0], in_=k2t[0])
        for l in range(3):
            v_load(l, 0)
        for c0 in range(512, T, 512):
            dma_cols(sb_kt[0][0], k0t[0], c0, c0 + 512)
        for c0 in (1536, 1024, 512):
            dma_cols(sb_qt[0], qt[0], c0, c0 + 512)
        # duo 1 (first needed at unit (1,3), mid-kernel)
        for c0 in range(0, T, 512):
            dma_cols(sb_kt[0][1], k0t[1], c0, c0 + 512)
        dma_cols(sb_qt[1], qt[1], 1536, 2048)
        for l in range(3):
            v_load(l, 1)
        for c0 in (1024, 512, 0):
            dma_cols(sb_qt[1], qt[1], c0, c0 + 512)
        nc.sync.dma_start(out=sb_kt[1][1], in_=k1t[1])
        nc.sync.dma_start(out=sb_kt[2][1], in_=k2t[1])

        def unit_jblocks(qb):
            return ([(0, j) for j in range(4 * qb + 4)]
                    + [(1, j) for j in range(NJ[1])]
                    + [(2, j) for j in range(NJ[2])])

        def unit_exp_order(qb):
            """Emission order + exp-engine assignment for one unit.

            L0 full blocks of qb>=1 go to DVE via Schraudolph approx exp
            (softmax weight noise ~1.7% there costs ~0.4% output rel-l2
            because those queries average over >=512 keys); qb=0 units
            offload their L1 blocks instead (keeps DVE busy at startup,
            ~0.3% rel-l2). Everything else stays exact on ACT. The two
            streams are interleaved so both engines ping-pong the 2 PSUM
            score buffers concurrently.
            """
            if qb >= 1:
                elig = [(0, j, h) for j in range(4 * qb) for h in range(2)]
            else:
                elig = [(1, j, h) for j in range(NJ[1]) for h in range(2)]
            skip = DVE_SKIP if qb >= 1 else 2
            dve = [it for i, it in enumerate(elig) if i % skip != skip - 1]
            dset = set(dve)
            act = [(l, j, h) for (l, j) in unit_jblocks(qb) for h in range(2)
                   if (l, j, h) not in dset]
            order = []
            na, nd = len(act), len(dve)
            ia = id_ = 0
            for _ in range(na + nd):
                # Bresenham-style proportional interleave of the two streams
                if id_ < nd and (ia >= na or id_ * na < ia * nd):
                    order.append((dve[id_], "V")); id_ += 1
                else:
                    order.append((act[ia], "A")); ia += 1
            return order

        def emit_block(d, qb, l, j, half, eng, pt_tiles):
                # ---- Phase 1 step: S^T = K^T.T @ Q^T for one (j, half),
                # then exp (ACT exact / DVE Schraudolph) -> P^T. One PSUM
                # bank per tile so 4 score buffers circulate concurrently.
                # Diagonal L0 blocks: columns left of the diagonal 128-col
                # sub-block are fully causal-masked and never read by PV,
                # so compute/exp only the [f0, 512) column range.
                f0 = 128 * (j - 4 * qb) if (l == 0 and j >= 4 * qb) else 0
                sp = qkp.tile([128, 512], F32, tag="qk", name="qk")
                nc.tensor.matmul(
                    out=sp[:, f0:],
                    lhsT=sb_kt[l][d][64 * half:64 * half + 64,
                                     128 * j:128 * j + 128],
                    rhs=sb_qt[d][64 * half:64 * half + 64,
                                 512 * qb + f0:512 * qb + 512],
                    start=True, stop=True,
                )
                pt = pts.tile([128, 512], BF16, tag="pt", name="pt")
                if eng == "V":
                    nc.vector.tensor_scalar(
                        out=pt.bitcast(I16), in0=sp,
                        scalar1=float(SCH_A), scalar2=float(SCH_B),
                        op0=Mult, op1=Add)
                else:
                    nc.scalar.activation(out=pt[:, f0:], in_=sp[:, f0:],
                                         func=Exp, scale=DH ** -0.5)
                if l == 0 and j >= 4 * qb:
                    s = pt[:, f0:f0 + 128]
                    nc.gpsimd.tensor_mul(out=s, in0=s, in1=sb_tri)
                pt_tiles[(l, j, half)] = pt

        def phase2_gen(d, qb, pt_tiles):
                jblocks = unit_jblocks(qb)
                # ---- Phase 2: PV accumulation + combine, per pair half.
                # A generator yielding every few matmuls so the driver can
                # interleave this PE-ready work between the next unit's
                # exp-gated QK blocks (keeps the in-order PE stream busy
                # while the exp engines drain).
                for half in range(2):
                    p = 2 * d + half
                    pvt = [pvp.tile([128, 4, 65], F32, tag="pv", name=f"pv{_l}") for _l in range(3)]
                    cnt = 0
                    for (l, j) in jblocks:
                        pt = pt_tiles[(l, j, half)]
                        # One accumulation group per PSUM bank (= per level):
                        # start zeroes the whole 2KB zero region, so only the
                        # very first matmul into the bank may set start=True.
                        lvl_last_j = (4 * qb + 3) if l == 0 else NJ[l] - 1
                        for c in range(4):
                            if l == 0 and j > 4 * qb + c:
                                continue
                            nc.tensor.matmul(
                                out=pvt[l][:, c, :],
                                lhsT=pt[:, 128 * c:128 * c + 128],
                                rhs=sb_v[l][:, p, j, :],
                                start=(j == 0 and c == 0),
                                stop=(j == lvl_last_j and c == 3),
                            )
                            cnt += 1
                            if cnt >= PV_CHUNK:
                                cnt = 0
                                yield
                    osb = outp.tile([128, 4, DH], F32, tag="osb", name="osb")
                    for l in range(3):
                        rc = small.tile([128, 4, 1], F32, tag="rc", name="rc")
                        nc.vector.reciprocal(out=rc[:, :, 0], in_=pvt[l][:, :, 64])
                        dst = osb if l == 0 else outp.tile([128, 4, DH], F32, tag="tmp", name="tmp")
                        nc.vector.tensor_mul(
                            out=dst, in0=pvt[l][:, :, 0:64],
                            in1=rc.broadcast_to([128, 4, DH]))
                        if l > 0:
                            nc.gpsimd.tensor_add(out=osb, in0=osb, in1=dst)
                    nc.sync.dma_start(out=out[p][:, 4 * qb:4 * qb + 4, :],
                                      in_=osb)
                    yield

        # Software pipeline: unit u's PV/combine stream is emitted interleaved
        # between unit u+1's QK+exp blocks (one generator chunk per block), so
        # the in-order PE always has satisfiable PV work queued ahead of each
        # exp-gated QK matmul. Unit order puts a small unit first (fast start
        # after partial DMA) and a small one last (short PV tail).
        units = [(0, 0), (0, 3), (0, 2), (0, 1), (1, 3), (1, 2), (1, 1), (1, 0)]
        gen = None
        for u in units[:-1]:
            d, qb = u
            tiles = {}
            for ((l, j, half), eng) in unit_exp_order(qb):
                emit_block(d, qb, l, j, half, eng, tiles)
                if gen is not None:
                    next(gen, None)
            if gen is not None:
                for _ in gen:
                    pass
            gen = phase2_gen(d, qb, tiles)
        # Last unit: emit half 0's blocks first, then start its own PV
        # generator during half 1's blocks (safe pumps only reach half-0
        # PV work), trimming the drain tail to half 1's PV + combine.
        d, qb = units[-1]
        tiles = {}
        order = unit_exp_order(qb)
        h0 = [it for it in order if it[0][2] == 0]
        h1 = [it for it in order if it[0][2] == 1]
        for ((l, j, half), eng) in h0:
            emit_block(d, qb, l, j, half, eng, tiles)
            if gen is not None:
                next(gen, None)
        for _ in gen:
            pass
        gen = phase2_gen(d, qb, tiles)
        safe = (16 * qb + 30) // PV_CHUNK + 1
        for i, ((l, j, half), eng) in enumerate(h1):
            emit_block(d, qb, l, j, half, eng, tiles)
            if safe > 0:
                next(gen, None)
                safe -= 1
        for _ in gen:
            pass
    nc.compile()
    return nc


def _prepare(inputs):
    bf = ml_dtypes.bfloat16
    Q = np.asarray(inputs["Q"], np.float32)
    Ks = [np.asarray(inputs[k], np.float32) for k in ("K0", "K1", "K2")]
    Vs = [np.asarray(inputs[k], np.float32) for k in ("V0", "V1", "V2")]
    level_w = np.asarray(inputs["level_w"], np.float64)
    e = np.exp(level_w - level_w.max())
    w = (e / e.sum()).astype(np.float64)

    # Host-side layout for sharding: per-head transposed Q/K ([64, Tm]) and
    # s-tiled V with a ones column ([128, nj, 65]).
    QT = np.ascontiguousarray(Q.transpose(0, 1, 3, 2)).astype(bf)  # [B,H,64,T]
    KTs = []
    for Kl in Ks:
        Tm = Kl.shape[1]
        Kh = Kl.reshape(B, Tm, H, DH).transpose(0, 2, 3, 1)  # [B,H,64,Tm]
        KTs.append(np.ascontiguousarray(Kh).astype(bf))
    Vps = []
    for lvl, Vl in enumerate(Vs):
        Tm = Vl.shape[1]
        Vl = Vl * np.float32(w[lvl])   # fold level weight into V (exact in fp32)
        Vh = Vl.reshape(B, Tm, H, DH).transpose(0, 2, 1, 3)  # [B,H,Tm,64]
        vp = np.ones((B, H, Tm // 128, 128, 65), np.float32)
        vp[..., :64] = Vh.reshape(B, H, Tm // 128, 128, DH)
        # -> [B, H, 128(p), nj, 65]
        Vps.append(np.ascontiguousarray(vp.transpose(0, 1, 3, 2, 4)).astype(bf))
    tri = (np.arange(128)[:, None] <= np.arange(128)[None, :]).astype(bf)

    in_maps = []
    for core in range(NCORES):
        m = {
            "qt": np.empty((DUOS, 128, T), bf),
            "k0t": np.empty((DUOS, 128, T), bf),
            "k1t": np.empty((DUOS, 128, T1), bf),
            "k2t": np.empty((DUOS, 128, T2), bf),
            "v0": np.empty((128, PAIRS, NJ[0], 65), bf),
            "v1": np.empty((128, PAIRS, NJ[1], 65), bf),
            "v2": np.empty((128, PAIRS, NJ[2], 65), bf),
            "tri": tri,
        }
        for p in range(PAIRS):
            g = PAIRS * core + p
            b, h = divmod(g, H)
            d, half = divmod(p, 2)
            sl = slice(64 * half, 64 * half + 64)
            m["qt"][d, sl] = QT[b, h]
            m["k0t"][d, sl] = KTs[0][b, h]
            m["k1t"][d, sl] = KTs[1][b, h]
            m["k2t"][d, sl] = KTs[2][b, h]
            m["v0"][:, p] = Vps[0][b, h]
            m["v1"][:, p] = Vps[1][b, h]
            m["v2"][:, p] = Vps[2][b, h]
        in_maps.append(m)

    return in_maps, w


def kernel(**inputs):
    global LAST_RESULT
    from concourse.bass_utils import run_bass_kernel_spmd

    in_maps, w = _prepare(inputs)
    key = tuple(np.asarray(w, np.float64).tolist())
    if key not in _NC_CACHE:
        _NC_CACHE[key] = _build_nc(w)
    nc = _NC_CACHE[key]

    try:
        res = run_bass_kernel_spmd(nc, in_maps, core_ids=list(range(NCORES)),
                                   trace=TRACE)
    except (ImportError, ModuleNotFoundError):
        # axon build without the NTFF profiling hook — run without trace
        res = run_bass_kernel_spmd(nc, in_maps, core_ids=list(range(NCORES)),
                                   trace=False)
    LAST_RESULT = res

    outs = np.stack([np.asarray(r["out"]) for r in res.results])  # [8,4,128,16,64]
    O = outs.transpose(0, 1, 3, 2, 4).reshape(B, H, T, DH)  # t = 128*n + pp
    return np.ascontiguousarray(O.transpose(0, 2, 1, 3)).reshape(B, T, C).astype(np.float32)

